# revision 1
# baseline (speedup 1.0000x reference)
"""Trainium2 Bass kernel for the pre-norm attention + SwiGLU FFN layer.

Sharding: tokens (batch*seq flattened) split across 8 cores — 512 tokens
each; cores 0-3 hold batch 0, cores 4-7 batch 1. All per-token work (LNs,
projections, rope, FFN) is fully local with replicated weights; attention
gathers the rope'd K and ones-padded V across each 4-core batch group with
one AllGather, then each core attends its 512 queries over the full 2048
context. The ones column appended to V makes the PV matmul emit softmax
denominators for free (row 64 of each head's PV output); softmax skips max
subtraction (scores are O(1) after QK-norm).

Weights are transposed host-side (numpy) so every matmul operand has the
contraction dim on partitions; matmuls run as float32r (full PE rate at
free-dim >= 256). rope cos/sin are host-expanded to per-token [T, D] tables
with the rotation sign folded in, so on-device rope is 2 strided copies +
3 elementwise ops.
"""

import os
import tempfile

import numpy as np

import bass_rust
import concourse.bass as bass
import concourse.mybir as mybir
import concourse.tile as tile
from concourse.bass_utils import run_bass_kernel_spmd
from concourse.masks import make_identity
from concourse.vector_clock import ScopedClock

F32 = mybir.dt.float32
F32R = mybir.dt.float32r
AF = mybir.ActivationFunctionType

N_CORES = 8
GROUP = 4
EPS = 1e-6

# ---------------------------------------------------------------------------
# Workaround for this walrus build's 1-wait-per-instruction encoding limit.
# ---------------------------------------------------------------------------
_MAX_WAITS = 1
_carrier_id = [0]


def _patched_drain_and_barrier(self, tick_clock, wait_clock):
    nc = self.nc
    drain_inst = nc.sync.drain()
    wait_clock.add_sem_waits(
        drain_inst.ins, ScopedClock({None: tick_clock.global_clock})
    )
    si = drain_inst.ins.sync_info
    waits = list(si.on_wait)
    if len(waits) > _MAX_WAITS:
        drain_inst.ins.sync_info = bass_rust.SyncInfo(
            on_wait=waits[:_MAX_WAITS], on_update=list(si.on_update)
        )
        rest = waits[_MAX_WAITS:]
        while rest:
            chunk, rest = rest[:_MAX_WAITS], rest[_MAX_WAITS:]
            extra = nc.sync.drain()
            extra.ins.sync_info = bass_rust.SyncInfo(on_wait=chunk, on_update=[])

    nc.all_engine_barrier()
    assert self.sems is not None
    popped = nc._tile_sem_poison_stack.pop()
    assert popped is self._sem_poison
    nc.clear_and_free_semaphores(list(self.sems.allocated().values()))
    nc.all_engine_barrier()


tile.TileContext._drain_and_barrier = _patched_drain_and_barrier


def _split_all_waits(nc, max_waits=_MAX_WAITS):
    for fn in nc.m.functions:
        for bb in fn.blocks:
            insts = list(bb.instructions)
            out = []
            changed = False
            for inst in insts:
                si = getattr(inst, "sync_info", None)
                if si is not None and si.on_wait and len(si.on_wait) > max_waits:
                    waits = list(si.on_wait)
                    updates = list(si.on_update)
                    extra, keep = waits[:-max_waits], waits[-max_waits:]
                    while extra:
                        chunk, extra = extra[:max_waits], extra[max_waits:]
                        _carrier_id[0] += 1
                        nop = mybir.InstNoOp(name=f"I-waitcar-{_carrier_id[0]}")
                        nop.engine = inst.engine
                        nop.sync_info = bass_rust.SyncInfo(on_wait=chunk, on_update=[])
                        nc.register_instruction(nop)
                        out.append(nop)
                    inst.sync_info = bass_rust.SyncInfo(on_wait=keep, on_update=updates)
                    changed = True
                out.append(inst)
            if changed:
                bb.instructions = out


# ---------------------------------------------------------------------------
# Graph builder (one SPMD program for all 8 cores)
# ---------------------------------------------------------------------------

def build_nc(T=512, D=1024, H=16, HD=64, FFN=4096, flags=frozenset()):
    """T: tokens per core; context = GROUP*T. flags: subset of
    {ln1_gb, qn_gb, kn_gb, ln2_gb, bqkv, bout, b1, b2, b3}."""
    NT = T // 128            # token tiles per core
    ND = D // 128            # model-dim tiles
    NH = FFN // 128          # ffn hidden tiles
    HP = H // 2              # head pairs (= ND)
    D3 = 3 * D
    NCH = D3 // 512          # qkv output chunks of 512
    VW = H * (HD + 1)        # padded v width per token (1040)
    KVF = D * T + T * VW     # floats in the per-core kv bounce

    nc = bass.Bass(trn_type="TRN2", num_devices=N_CORES)

    x_p = nc.declare_dram_parameter("x", [T, D], F32, isOutput=False)
    cos_p = nc.declare_dram_parameter("cosfull", [T, D], F32, isOutput=False)
    sin_p = nc.declare_dram_parameter("sinmod", [T, D], F32, isOutput=False)
    wqkv_p = nc.declare_dram_parameter("wqkvT", [D, D3], F32, isOutput=False)
    wout_p = nc.declare_dram_parameter("woutT", [D, D], F32, isOutput=False)
    w1_p = nc.declare_dram_parameter("w1T", [D, FFN], F32, isOutput=False)
    w3_p = nc.declare_dram_parameter("w3T", [D, FFN], F32, isOutput=False)
    w2_p = nc.declare_dram_parameter("w2T", [FFN, D], F32, isOutput=False)
    vecs = {}
    for name, size in [("ln1_g", D), ("ln1_b", D), ("qn_g", D), ("qn_b", D),
                       ("kn_g", D), ("kn_b", D), ("ln2_g", D), ("ln2_b", D),
                       ("b_qkv", D3), ("b_out", D), ("b1", FFN), ("b3", FFN),
                       ("b2", D)]:
        flag = {"ln1_g": "ln1_gb", "ln1_b": "ln1_gb", "qn_g": "qn_gb",
                "qn_b": "qn_gb", "kn_g": "kn_gb", "kn_b": "kn_gb",
                "ln2_g": "ln2_gb", "ln2_b": "ln2_gb", "b_qkv": "bqkv",
                "b_out": "bout", "b1": "b1", "b3": "b3", "b2": "b2"}[name]
        if flag in flags:
            vecs[name] = nc.declare_dram_parameter(name, [size], F32, isOutput=False)
    out_p = nc.declare_dram_parameter("out", [T, D], F32, isOutput=True)

    kv_in = nc.dram_tensor("kv_in", [KVF], F32)
    kv_all = nc.dram_tensor("kv_all", [GROUP * KVF], F32)

    def bcast_ap(param, width):
        return bass.AP(tensor=param.ap().tensor, offset=0,
                       ap=[[0, 128], [1, width]])

    from contextlib import ExitStack
    with tile.TileContext(nc) as tc, ExitStack() as stack:
        const = stack.enter_context(tc.tile_pool(name="const", bufs=1))
        ident = const.tile([128, 128], F32, tag="ident")
        make_identity(nc, ident)
        sel = const.tile([65, 128], F32, tag="sel")
        nc.vector.memset(sel, 0.0)
        nc.vector.memset(sel[64:65, :], 1.0)
        eps_t = const.tile([128, 1], F32, tag="eps")
        nc.vector.memset(eps_t, EPS)

        bc_tiles = {}
        for name in ("ln1_g", "ln1_b", "qn_g", "qn_b", "kn_g", "kn_b",
                     "ln2_g", "ln2_b", "b_out", "b2"):
            if name in vecs:
                t = const.tile([128, D], F32, tag=f"bc_{name}")
                nc.sync.dma_start(out=t, in_=bcast_ap(vecs[name], D))
                bc_tiles[name] = t
        if "b_qkv" in vecs:
            t = const.tile([128, D3], F32, tag="bc_bqkv")
            nc.sync.dma_start(out=t, in_=bcast_ap(vecs["b_qkv"], D3))
            bc_tiles["b_qkv"] = t
        for name in ("b1", "b3"):
            if name in vecs:
                # per-hidden scalars: [128, NH] with element (p, ht) = b[ht*128+p]
                t = const.tile([128, NH], F32, tag=f"col_{name}")
                ap = bass.AP(tensor=vecs[name].ap().tensor, offset=0,
                             ap=[[1, 128], [128, NH]])
                nc.sync.dma_start(out=t, in_=ap)
                bc_tiles[name] = t

        stat = stack.enter_context(tc.tile_pool(name="stat", bufs=4))
        xres = stack.enter_context(tc.tile_pool(name="xres", bufs=1))
        o1p = stack.enter_context(tc.tile_pool(name="o1p", bufs=1))

        x_N = [xres.tile([128, D], F32, tag=f"x{t}", name=f"x{t}") for t in range(NT)]
        out1_N = [o1p.tile([128, D], F32, tag=f"o1{t}", name=f"o1{t}") for t in range(NT)]

        def layer_norm_tiles(src_tile, dst_tile, gname):
            """dst = LN(src) with optional gain/bias, both [128, D]."""
            st = stat.tile([128, 2, 6], F32, tag="lnst")
            nc.vector.bn_stats(out=st[:, 0, :], in_=src_tile[:, 0:512])
            nc.vector.bn_stats(out=st[:, 1, :], in_=src_tile[:, 512:1024])
            mv = stat.tile([128, 2], F32, tag="lnmv")
            nc.vector.bn_aggr(out=mv, in_=st)
            rstd = stat.tile([128, 1], F32, tag="lnrstd")
            nc.scalar.activation(out=rstd, in_=mv[:, 1:2], func=AF.Sqrt,
                                 bias=eps_t, scale=1.0, alpha=0.0)
            nc.vector.reciprocal(out=rstd, in_=rstd)
            negmr = stat.tile([128, 1], F32, tag="lnnm")
            nc.vector.tensor_mul(out=negmr, in0=mv[:, 0:1], in1=rstd)
            nc.scalar.mul(out=negmr, in_=negmr, mul=-1.0)
            nc.scalar.activation(out=dst_tile, in_=src_tile, func=AF.Identity,
                                 scale=rstd, bias=negmr, alpha=0.0)
            if f"{gname}_g" in bc_tiles:
                nc.vector.tensor_mul(out=dst_tile, in0=dst_tile,
                                     in1=bc_tiles[f"{gname}_g"])
                nc.vector.tensor_add(out=dst_tile, in0=dst_tile,
                                     in1=bc_tiles[f"{gname}_b"])

        # ---- Phase A: load x, LN1, transpose h -> h_T --------------------
        qkv_res_cm = tc.tile_pool(name="qkv_res", bufs=1)
        qkv_res = qkv_res_cm.__enter__()
        q_T = [qkv_res.tile([128, T], F32, tag=f"qT{d}", name=f"qT{d}")
               for d in range(ND)]
        k_T = [qkv_res.tile([128, T], F32, tag=f"kT{d}", name=f"kT{d}")
               for d in range(ND)]
        v_pad = [qkv_res.tile([128, H, HD + 1], F32, tag=f"vp{t}", name=f"vp{t}")
                 for t in range(NT)]
        qknp_cm = tc.tile_pool(name="qknp", bufs=1)
        qknp = qknp_cm.__enter__()
        q_N = [qknp.tile([128, D], F32, tag=f"qN{t}", name=f"qN{t}") for t in range(NT)]
        k_N = [qknp.tile([128, D], F32, tag=f"kN{t}", name=f"kN{t}") for t in range(NT)]
        hTpool_cm = tc.tile_pool(name="hTpool", bufs=1)
        hTpool = hTpool_cm.__enter__()
        h_T = [hTpool.tile([128, T], F32, tag=f"hT{d}", name=f"hT{d}")
               for d in range(ND)]
        with (
            tc.tile_pool(name="hpool", bufs=2) as hpool,
            tc.tile_pool(name="trps", bufs=4, space="PSUM") as trps,
        ):
            for t in range(NT):
                nc.sync.dma_start(out=x_N[t], in_=x_p.ap()[t * 128:(t + 1) * 128, :])
                h_N = hpool.tile([128, D], F32, tag="hN")
                layer_norm_tiles(x_N[t], h_N, "ln1")
                for d in range(ND):
                    ptr = trps.tile([128, 128], F32, tag="trp")
                    nc.tensor.transpose(ptr, h_N[:, d * 128:(d + 1) * 128], ident)
                    nc.vector.tensor_copy(
                        out=h_T[d][:, t * 128:(t + 1) * 128].bitcast(F32R), in_=ptr)

        # ---- Phase B: QKV projection (h_T stationary, wT moving) ---------
        for t in range(NT):
            nc.vector.memset(v_pad[t][:, :, HD:HD + 1], 1.0)

        with (
            tc.tile_pool(name="wq", bufs=3) as wq,
            tc.tile_pool(name="mmps", bufs=8, space="PSUM") as mmps,
        ):
            for ch in range(NCH):
                ps = [mmps.tile([128, 512], F32, tag="qkvps", name=f"qkvps_{ch}_{t}") for t in range(NT)]
                for d in range(ND):
                    w = wq.tile([128, 512], F32, tag="wqt")
                    nc.sync.dma_start(
                        out=w.bitcast(F32R),
                        in_=wqkv_p.ap()[d * 128:(d + 1) * 128,
                                        ch * 512:(ch + 1) * 512].bitcast(F32R))
                    for t in range(NT):
                        nc.tensor.matmul(
                            ps[t], h_T[d][:, t * 128:(t + 1) * 128].bitcast(F32R),
                            w.bitcast(F32R), start=(d == 0), stop=(d == ND - 1))
                for t in range(NT):
                    if ch < 2:        # q chunks
                        dst = q_N[t][:, (ch % 2) * 512:(ch % 2) * 512 + 512]
                        src_bias = ("b_qkv", ch * 512)
                    elif ch < 4:      # k chunks
                        dst = k_N[t][:, (ch % 2) * 512:(ch % 2) * 512 + 512]
                        src_bias = ("b_qkv", ch * 512)
                    else:             # v chunks -> strided pad write
                        h0 = (ch - 4) * 8
                        dst = v_pad[t][:, h0:h0 + 8, 0:HD].bitcast(F32R)
                        if "b_qkv" in bc_tiles:
                            nc.vector.tensor_add(
                                out=dst,
                                in0=bc_tiles["b_qkv"][:, ch * 512:(ch + 1) * 512]
                                .rearrange("p (h f) -> p h f", h=8),
                                in1=ps[t].rearrange("p (h f) -> p h f", h=8))
                        else:
                            nc.vector.tensor_copy(
                                out=dst,
                                in_=ps[t].rearrange("p (h f) -> p h f", h=8))
                        continue
                    if "b_qkv" in bc_tiles:
                        nc.vector.tensor_add(
                            out=dst,
                            in0=bc_tiles["b_qkv"][:, src_bias[1]:src_bias[1] + 512],
                            in1=ps[t])
                    else:
                        nc.vector.tensor_copy(out=dst, in_=ps[t])

        # ---- Phase C: QK-norm + rope + transpose -------------------------

        def qknorm_stats(src_tile, gname):
            st = stat.tile([128, 2, 6], F32, tag="qkst")
            nc.vector.bn_stats(out=st[:, 0, :], in_=src_tile[:, 0:512])
            nc.vector.bn_stats(out=st[:, 1, :], in_=src_tile[:, 512:1024])
            mv = stat.tile([128, 2], F32, tag="qkmv")
            nc.vector.bn_aggr(out=mv, in_=st)
            rstd = stat.tile([128, 1], F32, tag="qkrstd")
            nc.scalar.activation(out=rstd, in_=mv[:, 1:2], func=AF.Sqrt,
                                 bias=eps_t, scale=1.0, alpha=0.0)
            nc.vector.reciprocal(out=rstd, in_=rstd)
            negmr = stat.tile([128, 1], F32, tag="qknm")
            nc.vector.tensor_mul(out=negmr, in0=mv[:, 0:1], in1=rstd)
            nc.scalar.mul(out=negmr, in_=negmr, mul=-1.0)
            return rstd, negmr

        with (
            tc.tile_pool(name="cspool", bufs=2) as cspool,
            tc.tile_pool(name="ropep", bufs=2) as ropep,
            tc.tile_pool(name="trps2", bufs=4, space="PSUM") as trps2,
        ):
            for t in range(NT):
                cosf = cspool.tile([128, D], F32, tag="cosf")
                sinm = cspool.tile([128, D], F32, tag="sinm")
                nc.sync.dma_start(out=cosf, in_=cos_p.ap()[t * 128:(t + 1) * 128, :])
                nc.sync.dma_start(out=sinm, in_=sin_p.ap()[t * 128:(t + 1) * 128, :])
                for which, src_N, dst_T, gname in (
                    ("q", q_N[t], q_T, "qn"), ("k", k_N[t], k_T, "kn"),
                ):
                    rstd, negmr = qknorm_stats(src_N, gname)
                    nrm = ropep.tile([128, D], F32, tag="nrm")
                    nc.scalar.activation(out=nrm, in_=src_N, func=AF.Identity,
                                         scale=rstd, bias=negmr, alpha=0.0)
                    if f"{gname}_g" in bc_tiles:
                        nc.vector.tensor_mul(out=nrm, in0=nrm,
                                             in1=bc_tiles[f"{gname}_g"])
                        nc.vector.tensor_add(out=nrm, in0=nrm,
                                             in1=bc_tiles[f"{gname}_b"])
                    nrm3 = nrm.rearrange("p (h f) -> p h f", h=H)
                    sw = ropep.tile([128, H, HD], F32, tag="sw")
                    nc.vector.tensor_copy(out=sw[:, :, 0:32], in_=nrm3[:, :, 32:64])
                    nc.vector.tensor_copy(out=sw[:, :, 32:64], in_=nrm3[:, :, 0:32])
                    swf = sw.rearrange("p h f -> p (h f)")
                    rp = ropep.tile([128, D], F32, tag="rp")
                    nc.vector.tensor_mul(out=rp, in0=nrm, in1=cosf)
                    nc.vector.tensor_mul(out=swf, in0=swf, in1=sinm)
                    nc.vector.tensor_add(out=rp, in0=rp, in1=swf)
                    for d in range(ND):
                        ptr = trps2.tile([128, 128], F32, tag="trp2")
                        nc.tensor.transpose(ptr, rp[:, d * 128:(d + 1) * 128], ident)
                        nc.vector.tensor_copy(
                            out=dst_T[d][:, t * 128:(t + 1) * 128].bitcast(F32R),
                            in_=ptr)

        hTpool_cm.__exit__(None, None, None)
        qknp_cm.__exit__(None, None, None)
        # ---- Phase D: bounce + grouped AllGather -------------------------
        for d in range(ND):
            dst = bass.AP(tensor=kv_in.ap().tensor, offset=d * 128 * T,
                          ap=[[T, 128], [1, T]])
            nc.sync.dma_start(out=dst.bitcast(F32R), in_=k_T[d].bitcast(F32R))
        voff = D * T
        for t in range(NT):
            dst = bass.AP(tensor=kv_in.ap().tensor, offset=voff + t * 128 * VW,
                          ap=[[VW, 128], [1, VW]])
            nc.sync.dma_start(out=dst.bitcast(F32R),
                              in_=v_pad[t].rearrange("p h f -> p (h f)").bitcast(F32R))
        groups = [list(range(g * GROUP, (g + 1) * GROUP))
                  for g in range(N_CORES // GROUP)]
        nc.gpsimd.collective_compute(
            "AllGather", mybir.AluOpType.bypass, replica_groups=groups,
            ins=[kv_in.ap().opt()], outs=[kv_all.ap().opt()])

        # ---- Phase E: attention ------------------------------------------
        attp_cm = tc.tile_pool(name="attp", bufs=1)
        attp = attp_cm.__enter__()
        accA = [attp.tile([65, T], F32, tag=f"accA{d}", name=f"accA{d}") for d in range(HP)]
        accB = [attp.tile([65, T], F32, tag=f"accB{d}", name=f"accB{d}") for d in range(HP)]
        stacked = [attp.tile([128, T], F32, tag=f"stk{d}", name=f"stk{d}") for d in range(HP)]

        KT_HALF = (GROUP * T // 128) // 2   # ktok tiles per half (8 full-size)
        CH_HALF = GROUP // 2                # rank chunks per half

        for ha in range(2):
            with (
                tc.tile_pool(name=f"vh{ha}", bufs=1) as vh,
                tc.tile_pool(name=f"kh{ha}", bufs=3) as kh,
                tc.tile_pool(name=f"scps{ha}", bufs=2, space="PSUM") as scps,
                tc.tile_pool(name=f"pvps{ha}", bufs=1, space="PSUM") as pvps,
                tc.tile_pool(name=f"prb{ha}", bufs=4) as prb,
            ):
                vtiles = []
                for i in range(KT_HALF):
                    rc = ha * CH_HALF + i // (T // 128)
                    st = i % (T // 128)
                    vt = vh.tile([128, VW], F32, tag=f"vt{i}", name=f"vt{i}_{ha}")
                    src = bass.AP(tensor=kv_all.ap().tensor,
                                  offset=rc * KVF + voff + st * 128 * VW,
                                  ap=[[VW, 128], [1, VW]])
                    nc.sync.dma_start(out=vt.bitcast(F32R), in_=src.bitcast(F32R))
                    vtiles.append(vt)
                for d in range(HP):
                    ks = []
                    for c2 in range(CH_HALF):
                        rc = ha * CH_HALF + c2
                        kt_ = kh.tile([128, T], F32, tag="kt")
                        src = bass.AP(tensor=kv_all.ap().tensor,
                                      offset=rc * KVF + d * 128 * T,
                                      ap=[[T, 128], [1, T]])
                        nc.sync.dma_start(out=kt_.bitcast(F32R),
                                          in_=src.bitcast(F32R))
                        ks.append(kt_)
                    pvA = pvps.tile([65, T], F32, tag="pvA")
                    pvB = pvps.tile([65, T], F32, tag="pvB")
                    hA, hB = 2 * d, 2 * d + 1
                    for kt in range(KT_HALF):
                        c2, st = divmod(kt, T // 128)
                        sl = slice(st * 128, (st + 1) * 128)
                        psA = scps.tile([128, T], F32, tag="psA")
                        psB = scps.tile([128, T], F32, tag="psB")
                        nc.tensor.matmul(psA, ks[c2][0:64, sl].bitcast(F32R),
                                         q_T[d][0:64, :].bitcast(F32R),
                                         start=True, stop=True,
                                         tile_position=(0, 0))
                        nc.tensor.matmul(psB, ks[c2][64:128, sl].bitcast(F32R),
                                         q_T[d][64:128, :].bitcast(F32R),
                                         start=True, stop=True,
                                         tile_position=(64, 0))
                        prA = prb.tile([128, T], F32, tag="prA")
                        prB = prb.tile([128, T], F32, tag="prB")
                        nc.scalar.activation(out=prA.bitcast(F32R), in_=psA,
                                             func=AF.Exp, scale=1.0 / np.sqrt(HD),
                                             alpha=0.0)
                        nc.scalar.activation(out=prB.bitcast(F32R), in_=psB,
                                             func=AF.Exp, scale=1.0 / np.sqrt(HD),
                                             alpha=0.0)
                        vt = vtiles[kt]
                        v3 = vt.rearrange("p (h f) -> p h f", h=H)
                        nc.tensor.matmul(pvA, v3[:, hA, :].bitcast(F32R),
                                         prA.bitcast(F32R),
                                         start=(kt == 0), stop=(kt == KT_HALF - 1))
                        nc.tensor.matmul(pvB, v3[:, hB, :].bitcast(F32R),
                                         prB.bitcast(F32R),
                                         start=(kt == 0), stop=(kt == KT_HALF - 1))
                    if ha == 0:
                        nc.vector.tensor_copy(out=accA[d].bitcast(F32R), in_=pvA)
                        nc.vector.tensor_copy(out=accB[d].bitcast(F32R), in_=pvB)
                    else:
                        nc.vector.tensor_add(out=accA[d].bitcast(F32R),
                                             in0=accA[d], in1=pvA)
                        nc.vector.tensor_add(out=accB[d].bitcast(F32R),
                                             in0=accB[d], in1=pvB)

        # scale by 1/denominator and restack head pairs
        with (
            tc.tile_pool(name="bcps", bufs=2, space="PSUM") as bcps,
            tc.tile_pool(name="tbp", bufs=2) as tbp,
        ):
            for d in range(HP):
                with nc.allow_low_precision(reason="f32r bits are f32"):
                    nc.vector.reciprocal(out=accA[d][64:65, :].bitcast(F32R),
                                         in_=accA[d][64:65, :])
                    nc.vector.reciprocal(out=accB[d][64:65, :].bitcast(F32R),
                                         in_=accB[d][64:65, :])
                bcA = bcps.tile([128, T], F32, tag="bcA")
                nc.tensor.matmul(bcA, sel.bitcast(F32R), accA[d].bitcast(F32R),
                                 start=True, stop=True)
                nc.vector.tensor_mul(out=stacked[d][0:64, :].bitcast(F32R),
                                     in0=accA[d][0:64, :], in1=bcA[0:64, :])
                bcB = bcps.tile([128, T], F32, tag="bcB")
                nc.tensor.matmul(bcB, sel.bitcast(F32R), accB[d].bitcast(F32R),
                                 start=True, stop=True)
                tmpB = tbp.tile([64, T], F32, tag="tmpB")
                nc.vector.tensor_mul(out=tmpB.bitcast(F32R),
                                     in0=accB[d][0:64, :], in1=bcB[0:64, :])
                nc.sync.dma_start(out=stacked[d][64:128, :].bitcast(F32R),
                                  in_=tmpB.bitcast(F32R))

        # ---- Phase F: out projection + residual --------------------------
        with (
            tc.tile_pool(name="wo", bufs=3) as wo,
            tc.tile_pool(name="ops", bufs=8, space="PSUM") as ops,
        ):
            for ch in range(D // 512):
                ps = [ops.tile([128, 512], F32, tag="ops", name=f"ops_{ch}_{t}") for t in range(NT)]
                for d in range(HP):
                    w = wo.tile([128, 512], F32, tag="wot")
                    nc.sync.dma_start(
                        out=w.bitcast(F32R),
                        in_=wout_p.ap()[d * 128:(d + 1) * 128,
                                        ch * 512:(ch + 1) * 512].bitcast(F32R))
                    for t in range(NT):
                        nc.tensor.matmul(
                            ps[t], stacked[d][:, t * 128:(t + 1) * 128].bitcast(F32R),
                            w.bitcast(F32R), start=(d == 0), stop=(d == HP - 1))
                for t in range(NT):
                    sl = slice(ch * 512, (ch + 1) * 512)
                    nc.vector.tensor_add(out=out1_N[t][:, sl],
                                         in0=x_N[t][:, sl], in1=ps[t])
                    if "b_out" in bc_tiles:
                        nc.vector.tensor_add(out=out1_N[t][:, sl],
                                             in0=out1_N[t][:, sl],
                                             in1=bc_tiles["b_out"][:, sl])

        attp_cm.__exit__(None, None, None)
        qkv_res_cm.__exit__(None, None, None)
        # ---- Phase G: LN2 + transpose ------------------------------------
        h2p = stack.enter_context(tc.tile_pool(name="h2p", bufs=1))
        h2_T = [h2p.tile([128, T], F32, tag=f"h2T{d}", name=f"h2T{d}") for d in range(ND)]
        with (
            tc.tile_pool(name="h2pool", bufs=2) as h2pool,
            tc.tile_pool(name="trps3", bufs=4, space="PSUM") as trps3,
        ):
            for t in range(NT):
                h2_N = h2pool.tile([128, D], F32, tag="h2N")
                layer_norm_tiles(out1_N[t], h2_N, "ln2")
                for d in range(ND):
                    ptr = trps3.tile([128, 128], F32, tag="trp3")
                    nc.tensor.transpose(ptr, h2_N[:, d * 128:(d + 1) * 128], ident)
                    nc.vector.tensor_copy(
                        out=h2_T[d][:, t * 128:(t + 1) * 128].bitcast(F32R), in_=ptr)

        # ---- Phase H: FFN -------------------------------------------------
        prp = stack.enter_context(tc.tile_pool(name="prp", bufs=1))
        prod_T = [prp.tile([128, T], F32, tag=f"pr{h}", name=f"pr{h}") for h in range(NH)]
        with (
            tc.tile_pool(name="wf", bufs=3) as wf,
            tc.tile_pool(name="ffps", bufs=2, space="PSUM") as ffps,
            tc.tile_pool(name="s1p", bufs=2) as s1p,
        ):
            for ht in range(NH):
                w1sb = wf.tile([128, ND, 128], F32, tag="w1sb")
                w3sb = wf.tile([128, ND, 128], F32, tag="w3sb")
                for wsb, wp in ((w1sb, w1_p), (w3sb, w3_p)):
                    src = bass.AP(tensor=wp.ap().tensor, offset=ht * 128,
                                  ap=[[FFN, 128], [128 * FFN, ND], [1, 128]])
                    nc.sync.dma_start(out=wsb.bitcast(F32R), in_=src.bitcast(F32R))
                ps1 = ffps.tile([128, T], F32, tag="ps1")
                ps3 = ffps.tile([128, T], F32, tag="ps3")
                for d in range(ND):
                    nc.tensor.matmul(ps1, w1sb[:, d, :].bitcast(F32R),
                                     h2_T[d].bitcast(F32R),
                                     start=(d == 0), stop=(d == ND - 1))
                for d in range(ND):
                    nc.tensor.matmul(ps3, w3sb[:, d, :].bitcast(F32R),
                                     h2_T[d].bitcast(F32R),
                                     start=(d == 0), stop=(d == ND - 1))
                s1 = s1p.tile([128, T], F32, tag="s1")
                b1arg = bc_tiles["b1"][:, ht:ht + 1] if "b1" in bc_tiles else 0.0
                nc.scalar.activation(out=s1, in_=ps1, func=AF.Silu,
                                     bias=b1arg, scale=1.0, alpha=0.0)
                if "b3" in bc_tiles:
                    t3 = s1p.tile([128, T], F32, tag="t3")
                    nc.vector.tensor_scalar_add(
                        out=t3, in0=ps3, scalar1=bc_tiles["b3"][:, ht:ht + 1])
                    nc.vector.tensor_mul(out=prod_T[ht].bitcast(F32R),
                                         in0=s1, in1=t3)
                else:
                    nc.vector.tensor_mul(out=prod_T[ht].bitcast(F32R),
                                         in0=s1, in1=ps3)

        with (
            tc.tile_pool(name="w2p", bufs=3) as w2p,
            tc.tile_pool(name="f2ps", bufs=8, space="PSUM") as f2ps,
            tc.tile_pool(name="finp", bufs=2) as finp,
        ):
            for ch in range(D // 512):
                ps = [f2ps.tile([128, 512], F32, tag="f2", name=f"f2_{ch}_{t}") for t in range(NT)]
                for ht in range(NH):
                    w = w2p.tile([128, 512], F32, tag="w2t")
                    nc.sync.dma_start(
                        out=w.bitcast(F32R),
                        in_=w2_p.ap()[ht * 128:(ht + 1) * 128,
                                      ch * 512:(ch + 1) * 512].bitcast(F32R))
                    for t in range(NT):
                        nc.tensor.matmul(
                            ps[t], prod_T[ht][:, t * 128:(t + 1) * 128].bitcast(F32R),
                            w.bitcast(F32R), start=(ht == 0), stop=(ht == NH - 1))
                for t in range(NT):
                    sl = slice(ch * 512, (ch + 1) * 512)
                    fin = finp.tile([128, 512], F32, tag="fin")
                    nc.vector.tensor_add(out=fin, in0=out1_N[t][:, sl], in1=ps[t])
                    if "b2" in bc_tiles:
                        nc.vector.tensor_add(out=fin, in0=fin,
                                             in1=bc_tiles["b2"][:, sl])
                    nc.sync.dma_start(out=out_p.ap()[t * 128:(t + 1) * 128, sl],
                                      in_=fin)

    _split_all_waits(nc)
    return nc


# ---------------------------------------------------------------------------
# Host wrapper
# ---------------------------------------------------------------------------

_CACHE = {}


def _prep_inputs(x, rope_cos, rope_sin, w_qkv, b_qkv, w_out, b_out,
                 qn_g, qn_b, kn_g, kn_b, ln1_g, ln1_b, ln2_g, ln2_b,
                 w1, b1, w2, b2, w3, b3):
    B, S, D = x.shape
    H, HD = 16, 64
    T = B * S // N_CORES

    flags = set()
    if not (np.all(ln1_g == 1) and np.all(ln1_b == 0)):
        flags.add("ln1_gb")
    if not (np.all(qn_g == 1) and np.all(qn_b == 0)):
        flags.add("qn_gb")
    if not (np.all(kn_g == 1) and np.all(kn_b == 0)):
        flags.add("kn_gb")
    if not (np.all(ln2_g == 1) and np.all(ln2_b == 0)):
        flags.add("ln2_gb")
    if np.any(b_qkv != 0):
        flags.add("bqkv")
    if np.any(b_out != 0):
        flags.add("bout")
    if np.any(b1 != 0):
        flags.add("b1")
    if np.any(b2 != 0):
        flags.add("b2")
    if np.any(b3 != 0):
        flags.add("b3")

    # host-side rope tables: [S, D] tiled over heads, rotation sign folded in
    cosfull = np.tile(rope_cos, (1, H)).astype(np.float32)          # [S, D]
    sinmod_half = np.concatenate(
        [-rope_sin[:, :HD // 2], rope_sin[:, HD // 2:]], axis=1)    # [S, HD]
    sinmod = np.tile(sinmod_half, (1, H)).astype(np.float32)        # [S, D]

    wqkvT = np.ascontiguousarray(w_qkv.T).astype(np.float32)
    woutT = np.ascontiguousarray(w_out.T).astype(np.float32)
    w1T = np.ascontiguousarray(w1.T).astype(np.float32)
    w3T = np.ascontiguousarray(w3.T).astype(np.float32)
    w2T = np.ascontiguousarray(w2.T).astype(np.float32)

    xf = np.ascontiguousarray(x.reshape(B * S, D)).astype(np.float32)
    in_maps = []
    for c in range(N_CORES):
        t0 = c * T
        m = {
            "x": xf[t0:t0 + T],
            "cosfull": np.ascontiguousarray(cosfull[t0 % S:t0 % S + T]),
            "sinmod": np.ascontiguousarray(sinmod[t0 % S:t0 % S + T]),
            "wqkvT": wqkvT, "woutT": woutT,
            "w1T": w1T, "w3T": w3T, "w2T": w2T,
        }
        opt = {"ln1_gb": [("ln1_g", ln1_g), ("ln1_b", ln1_b)],
               "qn_gb": [("qn_g", qn_g), ("qn_b", qn_b)],
               "kn_gb": [("kn_g", kn_g), ("kn_b", kn_b)],
               "ln2_gb": [("ln2_g", ln2_g), ("ln2_b", ln2_b)],
               "bqkv": [("b_qkv", b_qkv)], "bout": [("b_out", b_out)],
               "b1": [("b1", b1)], "b2": [("b2", b2)], "b3": [("b3", b3)]}
        for fl, items in opt.items():
            if fl in flags:
                for name, arr in items:
                    m[name] = np.ascontiguousarray(arr).astype(np.float32)
        in_maps.append(m)
    return in_maps, frozenset(flags), T, D


def kernel(**inputs):
    x = inputs["x"]
    B, S, D = x.shape
    in_maps, flags, T, _ = _prep_inputs(**inputs)

    key = (T, D, flags)
    if key not in _CACHE:
        _CACHE[key] = build_nc(T=T, D=D, flags=flags)
    nc = _CACHE[key]

    res = run_bass_kernel_spmd(nc, in_maps, core_ids=list(range(N_CORES)))
    out = np.empty((B * S, D), np.float32)
    for c in range(N_CORES):
        out[c * T:(c + 1) * T] = res.results[c]["out"]
    return out.reshape(B, S, D)



# revision 2
# speedup vs baseline: 1.0190x; 1.0190x over previous
"""Trainium2 Bass kernel v2: pre-norm attention + SwiGLU FFN layer.

Design: fully decoupled cores (NO collectives, no barrier). Tokens sharded
512/core (cores 0-3 batch 0, 4-7 batch 1); each core REDUNDANTLY computes
K,V for all 2048 tokens of its batch (+6.5 GFLOP) instead of all-gathering
them (-208us collective, no cross-core skew sensitivity). Per-core x is
reordered host-side so the core's own 512 tokens are always tiles 0-3
(attention is permutation-invariant over keys; rope tables are permuted
identically).

All matmuls bf16 (full PE rate, half SBUF/DMA), fp32 PSUM accumulation;
residual path fp32. All transposes are batched DMA-xbar transposes (one
instruction per [128,1024] tile) on the Activation HWDGE ring; all
weight/data loads are single big 3D-AP DMAs on the SP ring (per-DMA
dispatch costs ~0.6-1.2us serialized per ring, so instruction count on
each ring is minimized). The ones-column appended to V makes PV emit
softmax denominators for free; softmax skips max subtraction (scores O(1)
after QK-norm). bf16+tile_position crashes this HW (probe-verified), so
per-head K=64 QK contractions run as plain K=128 matmuls against
zero-masked q copies (qA/qB).
"""

import numpy as np
import ml_dtypes

import bass_rust
import concourse.bass as bass
import concourse.mybir as mybir
import concourse.tile as tile
from concourse.bass_utils import run_bass_kernel_spmd
from concourse.vector_clock import ScopedClock

F32 = mybir.dt.float32
BF16 = mybir.dt.bfloat16
AF = mybir.ActivationFunctionType
ALU = mybir.AluOpType

N_CORES = 8
GROUP = 4
EPS = 1e-6

# ---------------------------------------------------------------------------
# Workaround for this walrus build's 1-wait-per-instruction encoding limit.
# ---------------------------------------------------------------------------
_MAX_WAITS = 1
_carrier_id = [0]


def _patched_drain_and_barrier(self, tick_clock, wait_clock):
    nc = self.nc
    drain_inst = nc.sync.drain()
    wait_clock.add_sem_waits(
        drain_inst.ins, ScopedClock({None: tick_clock.global_clock})
    )
    si = drain_inst.ins.sync_info
    waits = list(si.on_wait)
    if len(waits) > _MAX_WAITS:
        drain_inst.ins.sync_info = bass_rust.SyncInfo(
            on_wait=waits[:_MAX_WAITS], on_update=list(si.on_update)
        )
        rest = waits[_MAX_WAITS:]
        while rest:
            chunk, rest = rest[:_MAX_WAITS], rest[_MAX_WAITS:]
            extra = nc.sync.drain()
            extra.ins.sync_info = bass_rust.SyncInfo(on_wait=chunk, on_update=[])

    nc.all_engine_barrier()
    assert self.sems is not None
    popped = nc._tile_sem_poison_stack.pop()
    assert popped is self._sem_poison
    nc.clear_and_free_semaphores(list(self.sems.allocated().values()))
    nc.all_engine_barrier()


tile.TileContext._drain_and_barrier = _patched_drain_and_barrier


def _split_all_waits(nc, max_waits=_MAX_WAITS):
    for fn in nc.m.functions:
        for bb in fn.blocks:
            insts = list(bb.instructions)
            out = []
            changed = False
            for inst in insts:
                si = getattr(inst, "sync_info", None)
                if si is not None and si.on_wait and len(si.on_wait) > max_waits:
                    waits = list(si.on_wait)
                    updates = list(si.on_update)
                    extra, keep = waits[:-max_waits], waits[-max_waits:]
                    while extra:
                        chunk, extra = extra[:max_waits], extra[max_waits:]
                        _carrier_id[0] += 1
                        nop = mybir.InstNoOp(name=f"I-waitcar-{_carrier_id[0]}")
                        nop.engine = inst.engine
                        nop.sync_info = bass_rust.SyncInfo(on_wait=chunk, on_update=[])
                        nc.register_instruction(nop)
                        out.append(nop)
                    inst.sync_info = bass_rust.SyncInfo(on_wait=keep, on_update=updates)
                    changed = True
                out.append(inst)
            if changed:
                bb.instructions = out


# ---------------------------------------------------------------------------
# Graph builder (one SPMD program; cores fully independent)
# ---------------------------------------------------------------------------

def build_nc(T=512, D=1024, H=16, HD=64, FFN=4096, flags=frozenset()):
    """T: own tokens per core; TF = GROUP*T batch context."""
    TF = GROUP * T           # full-batch tokens (2048)
    NT = T // 128            # own token tiles (4)
    NTF = TF // 128          # full token tiles (16)
    ND = D // 128            # model-dim tiles (8)
    NH = FFN // 128          # ffn hidden tiles (32)
    HP = H // 2              # head pairs (8)
    D3 = 3 * D

    nc = bass.Bass(trn_type="TRN2", num_devices=N_CORES)

    xo_p = nc.declare_dram_parameter("xo", [T, D], F32, isOutput=False)
    xf_p = nc.declare_dram_parameter("xf", [TF, D], BF16, isOutput=False)
    cos_p = nc.declare_dram_parameter("cosc", [TF, HD], BF16, isOutput=False)
    sin_p = nc.declare_dram_parameter("sinc", [TF, HD], BF16, isOutput=False)
    wqkv_p = nc.declare_dram_parameter("wqkvT", [D, D3], BF16, isOutput=False)
    wout_p = nc.declare_dram_parameter("woutT", [D, D], BF16, isOutput=False)
    w1_p = nc.declare_dram_parameter("w1T", [D, FFN], BF16, isOutput=False)
    w3_p = nc.declare_dram_parameter("w3T", [D, FFN], BF16, isOutput=False)
    w2_p = nc.declare_dram_parameter("w2T", [FFN, D], BF16, isOutput=False)
    vecs = {}
    for name, size in [("ln1_g", D), ("ln1_b", D), ("qn_g", D), ("qn_b", D),
                       ("kn_g", D), ("kn_b", D), ("ln2_g", D), ("ln2_b", D),
                       ("b_qkv", D3), ("b_out", D), ("b1", FFN), ("b3", FFN),
                       ("b2", D)]:
        flag = {"ln1_g": "ln1_gb", "ln1_b": "ln1_gb", "qn_g": "qn_gb",
                "qn_b": "qn_gb", "kn_g": "kn_gb", "kn_b": "kn_gb",
                "ln2_g": "ln2_gb", "ln2_b": "ln2_gb", "b_qkv": "bqkv",
                "b_out": "bout", "b1": "b1", "b3": "b3", "b2": "b2"}[name]
        if flag in flags:
            vecs[name] = nc.declare_dram_parameter(name, [size], F32, isOutput=False)
    out_p = nc.declare_dram_parameter("out", [T, D], F32, isOutput=True)

    def bcast_ap(param, width):
        return bass.AP(tensor=param.ap().tensor, offset=0,
                       ap=[[0, 128], [1, width]])

    from contextlib import ExitStack
    with tile.TileContext(nc) as tc, ExitStack() as stack:
        const = stack.enter_context(tc.tile_pool(name="const", bufs=1))
        sel = const.tile([65, 128], BF16, tag="sel")
        eps_t = const.tile([128, 1], F32, tag="eps")
        nc.vector.memset(eps_t, EPS)

        bc_tiles = {}
        for name in ("ln1_g", "ln1_b", "qn_g", "qn_b", "kn_g", "kn_b",
                     "ln2_g", "ln2_b", "b_out", "b2"):
            if name in vecs:
                t = const.tile([128, D], F32, tag=f"bc_{name}", name=f"bc_{name}")
                nc.sync.dma_start(out=t, in_=bcast_ap(vecs[name], D))
                bc_tiles[name] = t
        if "b_qkv" in vecs:
            t = const.tile([128, D3], F32, tag="bc_bqkv")
            nc.sync.dma_start(out=t, in_=bcast_ap(vecs["b_qkv"], D3))
            bc_tiles["b_qkv"] = t
        for name in ("b1", "b3"):
            if name in vecs:
                t = const.tile([128, NH], F32, tag=f"col_{name}", name=f"col_{name}")
                ap = bass.AP(tensor=vecs[name].ap().tensor, offset=0,
                             ap=[[1, 128], [128, NH]])
                nc.sync.dma_start(out=t, in_=ap)
                bc_tiles[name] = t

        stat = stack.enter_context(tc.tile_pool(name="stat", bufs=4))

        def ln_stats(src_tile):
            """mean/rstd of a [128, D] tile -> (rstd, negmr) [128,1] f32."""
            st = stat.tile([128, 2, 6], F32, tag="lnst", name="lnst")
            nc.vector.bn_stats(out=st[:, 0, :], in_=src_tile[:, 0:512])
            nc.vector.bn_stats(out=st[:, 1, :], in_=src_tile[:, 512:1024])
            mv = stat.tile([128, 2], F32, tag="lnmv", name="lnmv")
            nc.vector.bn_aggr(out=mv, in_=st)
            rstd = stat.tile([128, 1], F32, tag="lnrstd", name="lnrstd")
            nc.scalar.activation(out=rstd, in_=mv[:, 1:2], func=AF.Sqrt,
                                 bias=eps_t, scale=1.0, alpha=0.0)
            nc.vector.reciprocal(out=rstd, in_=rstd)
            negmr = stat.tile([128, 1], F32, tag="lnnm", name="lnnm")
            nc.vector.tensor_mul(out=negmr, in0=mv[:, 0:1], in1=rstd)
            nc.scalar.mul(out=negmr, in_=negmr, mul=-1.0)
            return rstd, negmr

        def ln_normalize(src_tile, dst_tile, gname):
            rstd, negmr = ln_stats(src_tile)
            nc.scalar.activation(out=dst_tile, in_=src_tile, func=AF.Identity,
                                 scale=rstd, bias=negmr, alpha=0.0)
            if f"{gname}_g" in bc_tiles:
                nc.vector.tensor_mul(out=dst_tile, in0=dst_tile,
                                     in1=bc_tiles[f"{gname}_g"])
                nc.vector.tensor_add(out=dst_tile, in0=dst_tile,
                                     in1=bc_tiles[f"{gname}_b"])

        # ---- persistent tiles (entered in LIFO order by lifetime end) ---
        o1_cm = tc.tile_pool(name="o1p", bufs=1)
        o1p = o1_cm.__enter__()
        out1 = [o1p.tile([128, D], F32, tag=f"o1{t}", name=f"o1{t}")
                for t in range(NT)]
        kqv_cm = tc.tile_pool(name="kqvp", bufs=1)
        kqvp = kqv_cm.__enter__()
        # k_T split per token tile so QK(kt) depends only on its own
        # transpose (a single shared tile serializes readers on ALL writers)
        k_T = [kqvp.tile([128, ND, 128], BF16, tag=f"kT{t}", name=f"kT{t}")
               for t in range(NTF)]
        q_A = kqvp.tile([128, ND, T], BF16, tag="qA", name="qA")
        q_B = kqvp.tile([128, ND, T], BF16, tag="qB", name="qB")
        v_pad = [kqvp.tile([128, H, HD + 1], BF16, tag=f"vp{t}", name=f"vp{t}")
                 for t in range(NTF)]
        cs_cm = tc.tile_pool(name="csp", bufs=1)
        csp = cs_cm.__enter__()
        cos_a = csp.tile([128, NTF, HD], BF16, tag="cs", name="cs")
        sin_a = csp.tile([128, NTF, HD], BF16, tag="sn", name="sn")
        hT_cm = tc.tile_pool(name="hTp", bufs=1)
        hTp = hT_cm.__enter__()
        h_T = [hTp.tile([128, ND, 128], BF16, tag=f"hT{t}", name=f"hT{t}")
               for t in range(NTF)]

        def ap3(param, offset, s0, n0, s1, n1, s2, n2):
            return bass.AP(tensor=param.ap().tensor, offset=offset,
                           ap=[[s0, n0], [s1, n1], [s2, n2]])

        kqn_cm = tc.tile_pool(name="kqn", bufs=1)
        kqn = kqn_cm.__enter__()
        q_N = [kqn.tile([128, D], BF16, tag=f"qN{t}", bufs=1, name="qN")
               for t in range(NT)]

        def ln_normalize_v(src_tile, dst_tile, gname):
            """LN on the vector engine (tensor_scalar (x*rstd)+negmr)."""
            rstd, negmr = ln_stats(src_tile)
            nc.vector.tensor_scalar(out=dst_tile, in0=src_tile,
                                    scalar1=rstd, scalar2=negmr,
                                    op0=ALU.mult, op1=ALU.add)
            if f"{gname}_g" in bc_tiles:
                nc.vector.tensor_mul(out=dst_tile, in0=dst_tile,
                                     in1=bc_tiles[f"{gname}_g"])
                nc.vector.tensor_add(out=dst_tile, in0=dst_tile,
                                     in1=bc_tiles[f"{gname}_b"])

        def rope_norm(src_N, gname, out_ap, t, ropep):
            """src_N [128, D] bf16 -> norm+rope -> batched transpose."""
            rstd, negmr = ln_stats(src_N)
            nrm = ropep.tile([128, D], BF16, tag="nrm", name="nrm")
            nc.scalar.activation(out=nrm, in_=src_N, func=AF.Identity,
                                 scale=rstd, bias=negmr, alpha=0.0)
            if f"{gname}_g" in bc_tiles:
                nc.vector.tensor_mul(out=nrm, in0=nrm,
                                     in1=bc_tiles[f"{gname}_g"])
                nc.vector.tensor_add(out=nrm, in0=nrm,
                                     in1=bc_tiles[f"{gname}_b"])
            nrm3 = nrm.rearrange("p (h f) -> p h f", h=H)
            # rope with the half-swap folded into shifted-slice reads:
            # sw[.., 0:32] = nrm[.., 32:64]*sin[0:32]; sw[.., 32:64] =
            # nrm[.., 0:32]*sin[32:64]; out = nrm*cos + sw
            sw = ropep.tile([128, H, HD], BF16, tag="sw", name="sw")
            cos_bc = cos_a[:, t, :].unsqueeze(1).broadcast_to((128, H, HD))
            sb0 = sin_a[:, t, 0:32].unsqueeze(1).broadcast_to((128, H, 32))
            sb1 = sin_a[:, t, 32:64].unsqueeze(1).broadcast_to((128, H, 32))
            nc.vector.tensor_mul(out=sw[:, :, 0:32], in0=nrm3[:, :, 32:64],
                                 in1=sb0)
            nc.vector.tensor_mul(out=sw[:, :, 32:64], in0=nrm3[:, :, 0:32],
                                 in1=sb1)
            nc.vector.tensor_mul(out=nrm3, in0=nrm3, in1=cos_bc)
            nc.vector.tensor_add(out=nrm3, in0=nrm3, in1=sw)
            nc.scalar.dma_start_transpose(out=out_ap, in_=nrm)

        with (
            tc.tile_pool(name="xfp", bufs=1) as xfp,
            tc.tile_pool(name="wq", bufs=4) as wq,
            tc.tile_pool(name="mmps", bufs=2, space="PSUM") as mmps,
            tc.tile_pool(name="ropep", bufs=2) as ropep,
        ):
            def wq_load(col0):
                """[128, ND, 512] slice of wqkvT columns col0..col0+512."""
                w = wq.tile([128, ND, 512], BF16, tag="wqkv", name="wqkv")
                nc.sync.dma_start(
                    out=w, in_=ap3(wqkv_p, col0, D3, 128, 128 * D3, ND, 1, 512))
                return w

            # ---- Phase A/B/C interleaved: per 4-tile slab: LN1 +
            # transpose, then the dependent proj wave ---------------------
            def lnA(s, slabs):
                """LN1 + transpose for a 2-tile slab of xf."""
                xs = xfp.tile([128, 2, D], BF16, tag="xs", bufs=2, name="xs")
                nc.sync.dma_start(
                    out=xs, in_=ap3(xf_p, s * 2 * 128 * D, D, 128, 128 * D, 2, 1, D))
                slabs.append(xs)
                for i in range(2):
                    t = s * 2 + i
                    hb = xfp.tile([128, D], BF16, tag="hb", bufs=2, name="hb")
                    ln_normalize_v(xs[:, i, :], hb, "ln1")
                    nc.scalar.dma_start_transpose(out=h_T[t], in_=hb)

            def proj_group(w, tiles):
                ps = [mmps.tile([128, 512], F32, tag=f"ps{i}", name=f"ps{i}")
                      for i in range(len(tiles))]
                for d in range(ND):
                    for i, t in enumerate(tiles):
                        nc.tensor.matmul(
                            ps[i], h_T[t][:, d, :], w[:, d, :],
                            start=(d == 0), stop=(d == ND - 1))
                return ps

            def qk_write(dst, half, ps, bias_off, engine):
                sl = slice(half * 512, half * 512 + 512)
                if "b_qkv" in bc_tiles:
                    nc.vector.tensor_add(
                        out=dst[:, sl],
                        in0=bc_tiles["b_qkv"][:, bias_off:bias_off + 512],
                        in1=ps)
                elif engine == "v":
                    nc.vector.tensor_copy(out=dst[:, sl], in_=ps)
                else:
                    nc.scalar.activation(out=dst[:, sl], in_=ps, func=AF.Copy,
                                         bias=0.0, scale=1.0, alpha=0.0)

            slabs = []
            lnA(0, slabs)
            lnA(1, slabs)
            wqk = [wq_load(0), wq_load(512), wq_load(D), wq_load(D + 512)]
            # cos/sin: single DMAs; cos_a[p, t, j] = cosc[t*128+p, j]
            nc.sync.dma_start(
                out=cos_a, in_=ap3(cos_p, 0, HD, 128, 128 * HD, NTF, 1, HD))
            nc.sync.dma_start(
                out=sin_a, in_=ap3(sin_p, 0, HD, 128, 128 * HD, NTF, 1, HD))

            # Q proj (own tiles 0-3) then Q norm+rope
            for ch in (0, 1):
                ps = proj_group(wqk[ch], list(range(NT)))
                for i in range(NT):
                    qk_write(q_N[i], ch, ps[i], ch * 512, "v")
            for t in range(NT):
                rope_norm(q_N[t], "qn",
                          q_A[:, :, t * 128:(t + 1) * 128], t, ropep)
            nc.vector.tensor_copy(out=q_B[64:128, :, :], in_=q_A[64:128, :, :])
            nc.vector.memset(q_A[64:128, :, :], 0.0)
            nc.vector.memset(q_B[0:64, :, :], 0.0)
            nc.vector.memset(sel, 0.0)
            nc.vector.memset(sel[64:65, :], 1.0)
            for t in range(NTF):
                nc.vector.memset(v_pad[t][:, :, HD:HD + 1], 1.0)

            # K proj in waves of 4 tiles, norm+rope chasing per wave;
            # LN1 for the next slab interleaves with each wave
            for tg in range(NTF // 4):
                if tg + 1 < NTF // 4:
                    lnA(2 * (tg + 1), slabs)
                    lnA(2 * (tg + 1) + 1, slabs)
                tiles = list(range(tg * 4, tg * 4 + 4))
                k_N = [kqn.tile([128, D], BF16, tag="kN", bufs=4, name="kN")
                       for _ in tiles]
                for ci in (0, 1):
                    ps = proj_group(wqk[2 + ci], tiles)
                    for i in range(4):
                        qk_write(k_N[i], ci, ps[i], D + ci * 512,
                                 "s" if ci == 0 else "v")
                for i, t in enumerate(tiles):
                    rope_norm(k_N[i], "kn", k_T[t], t, ropep)

            # V proj
            wv = [wq_load(2 * D + ch * 512) for ch in (0, 1)]
            for tg in range(NTF // 4):
                tiles = list(range(tg * 4, tg * 4 + 4))
                for ci, w in enumerate(wv):
                    ps = proj_group(w, tiles)
                    h0 = ci * 8
                    for i, t in enumerate(tiles):
                        dst = v_pad[t][:, h0:h0 + 8, 0:HD]
                        if "b_qkv" in bc_tiles:
                            nc.vector.tensor_add(
                                out=dst,
                                in0=bc_tiles["b_qkv"][:, 2 * D + ci * 512:
                                                      2 * D + ci * 512 + 512]
                                .rearrange("p (h f) -> p h f", h=8),
                                in1=ps[i].rearrange("p (h f) -> p h f", h=8))
                        else:
                            nc.vector.tensor_copy(
                                out=dst,
                                in_=ps[i].rearrange("p (h f) -> p h f", h=8))

        kqn_cm.__exit__(None, None, None)
        hT_cm.__exit__(None, None, None)
        cs_cm.__exit__(None, None, None)

        # ---- Phase E: attention -----------------------------------------
        attn_cm = tc.tile_pool(name="attnp", bufs=1)
        attnp = attn_cm.__enter__()
        attn_T = [attnp.tile([128, T], BF16, tag=f"at{d}", name=f"at{d}")
                  for d in range(ND)]
        scale = 1.0 / np.sqrt(HD)
        wo_cm = tc.tile_pool(name="wo", bufs=1)
        wo = wo_cm.__enter__()
        wob = wo.tile([128, ND, D], BF16, tag="wob", name="wob")
        nc.sync.dma_start(out=wob,
                          in_=ap3(wout_p, 0, D, 128, 128 * D, ND, 1, D))
        with (
            tc.tile_pool(name="scps", bufs=2, space="PSUM") as scps,
            tc.tile_pool(name="pvps", bufs=1, space="PSUM") as pvps,
            tc.tile_pool(name="bcps", bufs=1, space="PSUM") as bcps,
            tc.tile_pool(name="prb", bufs=24) as prb,
            tc.tile_pool(name="accp", bufs=2) as accp,
            tc.tile_pool(name="tbp", bufs=2) as tbp,
        ):
            prs = {}

            def emit_qk(d):
                """scores for head pair d: both heads in one 2-bank psum
                tile, one [128,1024] exp."""
                prs[d] = []
                for kt in range(NTF):
                    ps = scps.tile([128, 2, T], F32, tag="ps", name="ps")
                    nc.tensor.matmul(ps[:, 0, :], k_T[kt][:, d, :],
                                     q_A[:, d, :], start=True, stop=True)
                    nc.tensor.matmul(ps[:, 1, :], k_T[kt][:, d, :],
                                     q_B[:, d, :], start=True, stop=True)
                    pr = prb.tile([128, 2, T], BF16, tag="pr", name="pr")
                    nc.scalar.activation(out=pr, in_=ps, func=AF.Exp,
                                         scale=scale, alpha=0.0)
                    prs[d].append(pr)

            def emit_pv(d):
                pvA = pvps.tile([65, T], F32, tag="pvA", name="pvA")
                pvB = pvps.tile([65, T], F32, tag="pvB", name="pvB")
                hA, hB = 2 * d, 2 * d + 1
                for kt in range(NTF):
                    nc.tensor.matmul(pvA, v_pad[kt][:, hA, :],
                                     prs[d][kt][:, 0, :],
                                     start=(kt == 0), stop=(kt == NTF - 1))
                    nc.tensor.matmul(pvB, v_pad[kt][:, hB, :],
                                     prs[d][kt][:, 1, :],
                                     start=(kt == 0), stop=(kt == NTF - 1))
                accA = accp.tile([65, T], BF16, tag="accA", name="accA")
                accB = accp.tile([65, T], BF16, tag="accB", name="accB")
                nc.vector.tensor_copy(out=accA, in_=pvA)
                nc.vector.tensor_copy(out=accB, in_=pvB)
                bc = bcps.tile([128, 2, T], F32, tag="bc", name="bc")
                nc.tensor.matmul(bc[:, 0, :], sel, accA, start=True, stop=True)
                nc.tensor.matmul(bc[:, 1, :], sel, accB, start=True, stop=True)
                rc = accp.tile([128, 2, T], BF16, tag="rc", name="rc")
                with nc.allow_low_precision(reason="bf16 softmax denom"):
                    nc.vector.reciprocal(out=rc, in_=bc)
                nc.vector.tensor_mul(out=attn_T[d][0:64, :],
                                     in0=accA[0:64, :], in1=rc[0:64, 0, :])
                tmpB = tbp.tile([64, T], BF16, tag="tmpB", name="tmpB")
                nc.vector.tensor_mul(out=tmpB, in0=accB[0:64, :],
                                     in1=rc[0:64, 1, :])
                nc.scalar.dma_start(out=attn_T[d][64:128, :], in_=tmpB)
                del prs[d]

            emit_qk(0)
            for d in range(HP):
                if d + 1 < HP:
                    emit_qk(d + 1)
                emit_pv(d)

        # ---- Phase F+G fused: out proj + residual + LN2 + transpose -----
        pr_cm = tc.tile_pool(name="prp", bufs=1)
        prp = pr_cm.__enter__()
        prod_T = [prp.tile([128, T], BF16, tag=f"pr{h}", name=f"pr{h}")
                  for h in range(NH)]
        h2_cm = tc.tile_pool(name="h2p", bufs=1)
        h2p = h2_cm.__enter__()
        h2_T = h2p.tile([128, ND, T], BF16, tag="h2T", name="h2T")
        with (
            tc.tile_pool(name="xop", bufs=1) as xop,
            tc.tile_pool(name="ops", bufs=2, space="PSUM") as ops,
            tc.tile_pool(name="h2w", bufs=2) as h2w,
        ):
            xo_t = xop.tile([128, NT, D], F32, tag="xo", name="xo")
            nc.sync.dma_start(out=xo_t,
                              in_=ap3(xo_p, 0, D, 128, 128 * D, NT, 1, D))
            for t in range(NT):
                ps = [ops.tile([128, 512], F32, tag=f"op{i}", name=f"op{i}")
                      for i in range(2)]
                for ch in range(2):
                    for d in range(ND):
                        nc.tensor.matmul(
                            ps[ch], attn_T[d][:, t * 128:(t + 1) * 128],
                            wob[:, d, ch * 512:(ch + 1) * 512],
                            start=(d == 0), stop=(d == ND - 1))
                for ch in range(2):
                    sl = slice(ch * 512, (ch + 1) * 512)
                    nc.vector.tensor_add(out=out1[t][:, sl],
                                         in0=xo_t[:, t, sl], in1=ps[ch])
                    if "b_out" in bc_tiles:
                        nc.vector.tensor_add(out=out1[t][:, sl],
                                             in0=out1[t][:, sl],
                                             in1=bc_tiles["b_out"][:, sl])
                h2 = h2w.tile([128, D], BF16, tag="h2", name="h2")
                ln_normalize(out1[t], h2, "ln2")
                nc.scalar.dma_start_transpose(
                    out=h2_T[:, :, t * 128:(t + 1) * 128], in_=h2)

        # ---- Phase H: FFN. FFN2's ch0 accumulation interleaves with FFN1
        # (4 psum banks each); ch1 runs as a dense second pass. ------------
        with (
            tc.tile_pool(name="wf", bufs=2) as wf,
            tc.tile_pool(name="w2p", bufs=2) as w2p,
            tc.tile_pool(name="ffps", bufs=2, space="PSUM") as ffps,
            tc.tile_pool(name="f2ps", bufs=1, space="PSUM") as f2ps,
            tc.tile_pool(name="s1p", bufs=2) as s1p,
            tc.tile_pool(name="finp", bufs=2) as finp,
        ):
            ps2 = [f2ps.tile([128, 512], F32, tag=f"f2{i}", name=f"f2{i}")
                   for i in range(NT)]

            def w2_load(hg, ch):
                w2b = w2p.tile([128, 4, 512], BF16, tag="w2b", name="w2b")
                nc.sync.dma_start(
                    out=w2b,
                    in_=ap3(w2_p, hg * 4 * 128 * D + ch * 512,
                            D, 128, 128 * D, 4, 1, 512))
                return w2b

            # FFN2-ch0 mms lag FFN1 by one ht so the PE never waits on the
            # vector mul that produces prod_T[ht]
            pending = []

            def flush_pending():
                for ht, w2b in pending:
                    for t in range(NT):
                        nc.tensor.matmul(
                            ps2[t], prod_T[ht][:, t * 128:(t + 1) * 128],
                            w2b[:, ht % 4, :],
                            start=(ht == 0), stop=(ht == NH - 1))
                pending.clear()

            for hg in range(NH // 4):
                w1b = wf.tile([128, ND, 512], BF16, tag="w1b", name="w1b")
                w3b = wf.tile([128, ND, 512], BF16, tag="w3b", name="w3b")
                nc.sync.dma_start(
                    out=w1b, in_=ap3(w1_p, hg * 512, FFN, 128, 128 * FFN, ND, 1, 512))
                nc.sync.dma_start(
                    out=w3b, in_=ap3(w3_p, hg * 512, FFN, 128, 128 * FFN, ND, 1, 512))
                w2b = w2_load(hg, 0)
                for i in range(4):
                    ht = hg * 4 + i
                    hsl = slice(i * 128, (i + 1) * 128)
                    ps1 = ffps.tile([128, T], F32, tag="ps1", name="ps1")
                    ps3 = ffps.tile([128, T], F32, tag="ps3", name="ps3")
                    for d in range(ND):
                        nc.tensor.matmul(ps1, w1b[:, d, hsl], h2_T[:, d, :],
                                         start=(d == 0), stop=(d == ND - 1))
                    flush_pending()
                    for d in range(ND):
                        nc.tensor.matmul(ps3, w3b[:, d, hsl], h2_T[:, d, :],
                                         start=(d == 0), stop=(d == ND - 1))
                    s1 = s1p.tile([128, T], BF16, tag="s1", name="s1")
                    b1arg = (bc_tiles["b1"][:, ht:ht + 1]
                             if "b1" in bc_tiles else 0.0)
                    nc.scalar.activation(out=s1, in_=ps1, func=AF.Silu,
                                         bias=b1arg, scale=1.0, alpha=0.0)
                    if "b3" in bc_tiles:
                        t3 = s1p.tile([128, T], F32, tag="t3", name="t3")
                        nc.vector.tensor_scalar_add(
                            out=t3, in0=ps3,
                            scalar1=bc_tiles["b3"][:, ht:ht + 1])
                        nc.vector.tensor_mul(out=prod_T[ht], in0=s1, in1=t3)
                    else:
                        nc.vector.tensor_mul(out=prod_T[ht], in0=s1, in1=ps3)
                    pending.append((ht, w2b))
            flush_pending()
            for t in range(NT):
                fin = finp.tile([128, 512], F32, tag="fin", name="fin")
                nc.vector.tensor_add(out=fin, in0=out1[t][:, 0:512],
                                     in1=ps2[t])
                if "b2" in bc_tiles:
                    nc.vector.tensor_add(out=fin, in0=fin,
                                         in1=bc_tiles["b2"][:, 0:512])
                nc.sync.dma_start(out=out_p.ap()[t * 128:(t + 1) * 128, 0:512],
                                  in_=fin)
            # ch1 second pass over stored prod_T (w2 ch1 halves re-loaded)
            psb = [ffps.tile([128, T], F32, tag="ps1", name="psb1"),
                   ffps.tile([128, T], F32, tag="ps3", name="psb3"),
                   ffps.tile([128, T], F32, tag="ps1", name="psb1b"),
                   ffps.tile([128, T], F32, tag="ps3", name="psb3b")]
            for hg in range(NH // 4):
                w2c = w2_load(hg, 1)
                for i in range(4):
                    ht = hg * 4 + i
                    for t in range(NT):
                        nc.tensor.matmul(
                            psb[t], prod_T[ht][:, t * 128:(t + 1) * 128],
                            w2c[:, i, :],
                            start=(ht == 0), stop=(ht == NH - 1))
            for t in range(NT):
                fin = finp.tile([128, 512], F32, tag="fin", name="fin")
                nc.vector.tensor_add(out=fin, in0=out1[t][:, 512:1024],
                                     in1=psb[t])
                if "b2" in bc_tiles:
                    nc.vector.tensor_add(out=fin, in0=fin,
                                         in1=bc_tiles["b2"][:, 512:1024])
                nc.sync.dma_start(
                    out=out_p.ap()[t * 128:(t + 1) * 128, 512:1024], in_=fin)

        h2_cm.__exit__(None, None, None)

        pr_cm.__exit__(None, None, None)
        wo_cm.__exit__(None, None, None)
        attn_cm.__exit__(None, None, None)
        kqv_cm.__exit__(None, None, None)
        o1_cm.__exit__(None, None, None)

    _split_all_waits(nc)
    return nc


# ---------------------------------------------------------------------------
# Host wrapper
# ---------------------------------------------------------------------------

_CACHE = {}
BF = ml_dtypes.bfloat16


def _prep_inputs(x, rope_cos, rope_sin, w_qkv, b_qkv, w_out, b_out,
                 qn_g, qn_b, kn_g, kn_b, ln1_g, ln1_b, ln2_g, ln2_b,
                 w1, b1, w2, b2, w3, b3):
    B, S, D = x.shape
    H, HD = 16, 64
    T = B * S // N_CORES

    flags = set()
    if not (np.all(ln1_g == 1) and np.all(ln1_b == 0)):
        flags.add("ln1_gb")
    if not (np.all(qn_g == 1) and np.all(qn_b == 0)):
        flags.add("qn_gb")
    if not (np.all(kn_g == 1) and np.all(kn_b == 0)):
        flags.add("kn_gb")
    if not (np.all(ln2_g == 1) and np.all(ln2_b == 0)):
        flags.add("ln2_gb")
    if np.any(b_qkv != 0):
        flags.add("bqkv")
    if np.any(b_out != 0):
        flags.add("bout")
    if np.any(b1 != 0):
        flags.add("b1")
    if np.any(b2 != 0):
        flags.add("b2")
    if np.any(b3 != 0):
        flags.add("b3")

    # compact rope tables with the rotation sign folded into sin
    sinmod = np.concatenate(
        [-rope_sin[:, :HD // 2], rope_sin[:, HD // 2:]], axis=1)  # [S, HD]
    cosc = np.asarray(rope_cos, np.float32)

    wqkvT = np.ascontiguousarray(w_qkv.T).astype(BF)
    woutT = np.ascontiguousarray(w_out.T).astype(BF)
    w1T = np.ascontiguousarray(w1.T).astype(BF)
    w3T = np.ascontiguousarray(w3.T).astype(BF)
    w2T = np.ascontiguousarray(w2.T).astype(BF)

    in_maps = []
    for c in range(N_CORES):
        b, qt = divmod(c, GROUP)
        o0 = qt * T
        perm = np.concatenate([np.arange(o0, o0 + T),
                               np.arange(0, o0),
                               np.arange(o0 + T, S)])
        xb = np.asarray(x[b], np.float32)
        m = {
            "xo": np.ascontiguousarray(xb[o0:o0 + T]),
            "xf": np.ascontiguousarray(xb[perm]).astype(BF),
            "cosc": np.ascontiguousarray(cosc[perm]).astype(BF),
            "sinc": np.ascontiguousarray(sinmod[perm]).astype(BF),
            "wqkvT": wqkvT, "woutT": woutT,
            "w1T": w1T, "w3T": w3T, "w2T": w2T,
        }
        opt = {"ln1_gb": [("ln1_g", ln1_g), ("ln1_b", ln1_b)],
               "qn_gb": [("qn_g", qn_g), ("qn_b", qn_b)],
               "kn_gb": [("kn_g", kn_g), ("kn_b", kn_b)],
               "ln2_gb": [("ln2_g", ln2_g), ("ln2_b", ln2_b)],
               "bqkv": [("b_qkv", b_qkv)], "bout": [("b_out", b_out)],
               "b1": [("b1", b1)], "b2": [("b2", b2)], "b3": [("b3", b3)]}
        for fl, items in opt.items():
            if fl in flags:
                for name, arr in items:
                    m[name] = np.ascontiguousarray(arr).astype(np.float32)
        in_maps.append(m)
    return in_maps, frozenset(flags), T, D


def kernel(**inputs):
    x = inputs["x"]
    B, S, D = x.shape
    in_maps, flags, T, _ = _prep_inputs(**inputs)

    key = (T, D, flags)
    if key not in _CACHE:
        _CACHE[key] = build_nc(T=T, D=D, flags=flags)
    nc = _CACHE[key]

    res = run_bass_kernel_spmd(nc, in_maps, core_ids=list(range(N_CORES)))
    out = np.empty((B * S, D), np.float32)
    for c in range(N_CORES):
        out[c * T:(c + 1) * T] = res.results[c]["out"]
    return out.reshape(B, S, D)


# revision 4
# speedup vs baseline: 1.0278x; 1.0086x over previous
"""Trainium2 Bass kernel v2: pre-norm attention + SwiGLU FFN layer.

Design: fully decoupled cores (NO collectives, no barrier). Tokens sharded
512/core (cores 0-3 batch 0, 4-7 batch 1); each core REDUNDANTLY computes
K,V for all 2048 tokens of its batch (+6.5 GFLOP) instead of all-gathering
them (-208us collective, no cross-core skew sensitivity). Per-core x is
reordered host-side so the core's own 512 tokens are always tiles 0-3
(attention is permutation-invariant over keys; rope tables are permuted
identically).

All matmuls bf16 (full PE rate, half SBUF/DMA), fp32 PSUM accumulation;
residual path fp32. All transposes are batched DMA-xbar transposes (one
instruction per [128,1024] tile) on the Activation HWDGE ring; all
weight/data loads are single big 3D-AP DMAs on the SP ring (per-DMA
dispatch costs ~0.6-1.2us serialized per ring, so instruction count on
each ring is minimized). The ones-column appended to V makes PV emit
softmax denominators for free; softmax skips max subtraction (scores O(1)
after QK-norm). bf16+tile_position crashes this HW (probe-verified), so
per-head K=64 QK contractions run as plain K=128 matmuls against
zero-masked q copies (qA/qB).
"""

import numpy as np
import ml_dtypes

import bass_rust
import concourse.bass as bass
import concourse.mybir as mybir
import concourse.tile as tile
from concourse.bass_utils import run_bass_kernel_spmd
from concourse.vector_clock import ScopedClock

F32 = mybir.dt.float32
BF16 = mybir.dt.bfloat16
AF = mybir.ActivationFunctionType
ALU = mybir.AluOpType

N_CORES = 8
GROUP = 4
EPS = 1e-6

# ---------------------------------------------------------------------------
# Workaround for this walrus build's 1-wait-per-instruction encoding limit.
# ---------------------------------------------------------------------------
_MAX_WAITS = 1
_carrier_id = [0]


def _patched_drain_and_barrier(self, tick_clock, wait_clock):
    nc = self.nc
    drain_inst = nc.sync.drain()
    wait_clock.add_sem_waits(
        drain_inst.ins, ScopedClock({None: tick_clock.global_clock})
    )
    si = drain_inst.ins.sync_info
    waits = list(si.on_wait)
    if len(waits) > _MAX_WAITS:
        drain_inst.ins.sync_info = bass_rust.SyncInfo(
            on_wait=waits[:_MAX_WAITS], on_update=list(si.on_update)
        )
        rest = waits[_MAX_WAITS:]
        while rest:
            chunk, rest = rest[:_MAX_WAITS], rest[_MAX_WAITS:]
            extra = nc.sync.drain()
            extra.ins.sync_info = bass_rust.SyncInfo(on_wait=chunk, on_update=[])

    nc.all_engine_barrier()
    assert self.sems is not None
    popped = nc._tile_sem_poison_stack.pop()
    assert popped is self._sem_poison
    nc.clear_and_free_semaphores(list(self.sems.allocated().values()))
    nc.all_engine_barrier()


tile.TileContext._drain_and_barrier = _patched_drain_and_barrier


def _split_all_waits(nc, max_waits=_MAX_WAITS):
    for fn in nc.m.functions:
        for bb in fn.blocks:
            insts = list(bb.instructions)
            out = []
            changed = False
            for inst in insts:
                si = getattr(inst, "sync_info", None)
                if si is not None and si.on_wait and len(si.on_wait) > max_waits:
                    waits = list(si.on_wait)
                    updates = list(si.on_update)
                    extra, keep = waits[:-max_waits], waits[-max_waits:]
                    while extra:
                        chunk, extra = extra[:max_waits], extra[max_waits:]
                        _carrier_id[0] += 1
                        nop = mybir.InstNoOp(name=f"I-waitcar-{_carrier_id[0]}")
                        nop.engine = inst.engine
                        nop.sync_info = bass_rust.SyncInfo(on_wait=chunk, on_update=[])
                        nc.register_instruction(nop)
                        out.append(nop)
                    inst.sync_info = bass_rust.SyncInfo(on_wait=keep, on_update=updates)
                    changed = True
                out.append(inst)
            if changed:
                bb.instructions = out


# ---------------------------------------------------------------------------
# Graph builder (one SPMD program; cores fully independent)
# ---------------------------------------------------------------------------

def build_nc(T=512, D=1024, H=16, HD=64, FFN=4096, flags=frozenset()):
    """T: own tokens per core; TF = GROUP*T batch context."""
    TF = GROUP * T           # full-batch tokens (2048)
    NT = T // 128            # own token tiles (4)
    NTF = TF // 128          # full token tiles (16)
    ND = D // 128            # model-dim tiles (8)
    NH = FFN // 128          # ffn hidden tiles (32)
    HP = H // 2              # head pairs (8)
    D3 = 3 * D

    nc = bass.Bass(trn_type="TRN2", num_devices=N_CORES)

    xo_p = nc.declare_dram_parameter("xo", [T, D], F32, isOutput=False)
    xf_p = nc.declare_dram_parameter("xf", [TF, D], BF16, isOutput=False)
    cos_p = nc.declare_dram_parameter("cosc", [TF, HD], BF16, isOutput=False)
    sin_p = nc.declare_dram_parameter("sinc", [TF, HD], BF16, isOutput=False)
    wqkv_p = nc.declare_dram_parameter("wqkvT", [D, D3], BF16, isOutput=False)
    wout_p = nc.declare_dram_parameter("woutT", [D, D], BF16, isOutput=False)
    w1_p = nc.declare_dram_parameter("w1T", [D, FFN], BF16, isOutput=False)
    w3_p = nc.declare_dram_parameter("w3T", [D, FFN], BF16, isOutput=False)
    w2_p = nc.declare_dram_parameter("w2T", [FFN, D], BF16, isOutput=False)
    vecs = {}
    for name, size in [("ln1_g", D), ("ln1_b", D), ("qn_g", D), ("qn_b", D),
                       ("kn_g", D), ("kn_b", D), ("ln2_g", D), ("ln2_b", D),
                       ("b_qkv", D3), ("b_out", D), ("b1", FFN), ("b3", FFN),
                       ("b2", D)]:
        flag = {"ln1_g": "ln1_gb", "ln1_b": "ln1_gb", "qn_g": "qn_gb",
                "qn_b": "qn_gb", "kn_g": "kn_gb", "kn_b": "kn_gb",
                "ln2_g": "ln2_gb", "ln2_b": "ln2_gb", "b_qkv": "bqkv",
                "b_out": "bout", "b1": "b1", "b3": "b3", "b2": "b2"}[name]
        if flag in flags:
            vecs[name] = nc.declare_dram_parameter(name, [size], F32, isOutput=False)
    out_p = nc.declare_dram_parameter("out", [T, D], F32, isOutput=True)

    def bcast_ap(param, width):
        return bass.AP(tensor=param.ap().tensor, offset=0,
                       ap=[[0, 128], [1, width]])

    from contextlib import ExitStack
    with tile.TileContext(nc) as tc, ExitStack() as stack:
        const = stack.enter_context(tc.tile_pool(name="const", bufs=1))
        sel = const.tile([65, 128], BF16, tag="sel")
        eps_t = const.tile([128, 1], F32, tag="eps")
        nc.vector.memset(eps_t, EPS)

        bc_tiles = {}
        for name in ("ln1_g", "ln1_b", "qn_g", "qn_b", "kn_g", "kn_b",
                     "ln2_g", "ln2_b", "b_out", "b2"):
            if name in vecs:
                t = const.tile([128, D], F32, tag=f"bc_{name}", name=f"bc_{name}")
                nc.sync.dma_start(out=t, in_=bcast_ap(vecs[name], D))
                bc_tiles[name] = t
        if "b_qkv" in vecs:
            t = const.tile([128, D3], F32, tag="bc_bqkv")
            nc.sync.dma_start(out=t, in_=bcast_ap(vecs["b_qkv"], D3))
            bc_tiles["b_qkv"] = t
        for name in ("b1", "b3"):
            if name in vecs:
                t = const.tile([128, NH], F32, tag=f"col_{name}", name=f"col_{name}")
                ap = bass.AP(tensor=vecs[name].ap().tensor, offset=0,
                             ap=[[1, 128], [128, NH]])
                nc.sync.dma_start(out=t, in_=ap)
                bc_tiles[name] = t

        stat = stack.enter_context(tc.tile_pool(name="stat", bufs=4))

        def ln_stats(src_tile):
            """mean/rstd of a [128, D] tile -> (rstd, negmr) [128,1] f32."""
            st = stat.tile([128, 2, 6], F32, tag="lnst", name="lnst")
            nc.vector.bn_stats(out=st[:, 0, :], in_=src_tile[:, 0:512])
            nc.vector.bn_stats(out=st[:, 1, :], in_=src_tile[:, 512:1024])
            mv = stat.tile([128, 2], F32, tag="lnmv", name="lnmv")
            nc.vector.bn_aggr(out=mv, in_=st)
            rstd = stat.tile([128, 1], F32, tag="lnrstd", name="lnrstd")
            nc.scalar.activation(out=rstd, in_=mv[:, 1:2], func=AF.Sqrt,
                                 bias=eps_t, scale=1.0, alpha=0.0)
            nc.vector.reciprocal(out=rstd, in_=rstd)
            mr = stat.tile([128, 1], F32, tag="lnmr", name="lnmr")
            nc.vector.tensor_mul(out=mr, in0=mv[:, 0:1], in1=rstd)
            return rstd, mr

        def ln_normalize(src_tile, dst_tile, gname):
            """LN via vector tensor_scalar: dst = src*rstd - mean*rstd."""
            rstd, mr = ln_stats(src_tile)
            nc.vector.tensor_scalar(out=dst_tile, in0=src_tile,
                                    scalar1=rstd, scalar2=mr,
                                    op0=ALU.mult, op1=ALU.subtract)
            if f"{gname}_g" in bc_tiles:
                nc.vector.tensor_mul(out=dst_tile, in0=dst_tile,
                                     in1=bc_tiles[f"{gname}_g"])
                nc.vector.tensor_add(out=dst_tile, in0=dst_tile,
                                     in1=bc_tiles[f"{gname}_b"])

        # ---- persistent tiles (entered in LIFO order by lifetime end) ---
        o1_cm = tc.tile_pool(name="o1p", bufs=1)
        o1p = o1_cm.__enter__()
        out1 = [o1p.tile([128, D], F32, tag=f"o1{t}", name=f"o1{t}")
                for t in range(NT)]
        kqv_cm = tc.tile_pool(name="kqvp", bufs=1)
        kqvp = kqv_cm.__enter__()
        # k_T split per token tile so QK(kt) depends only on its own
        # transpose (a single shared tile serializes readers on ALL writers)
        k_T = [kqvp.tile([128, ND, 128], BF16, tag=f"kT{t}", name=f"kT{t}")
               for t in range(NTF)]
        q_A = kqvp.tile([128, ND, T], BF16, tag="qA", name="qA")
        q_B = kqvp.tile([128, ND, T], BF16, tag="qB", name="qB")
        v_pad = [kqvp.tile([128, H, HD + 1], BF16, tag=f"vp{t}", name=f"vp{t}")
                 for t in range(NTF)]
        cs_cm = tc.tile_pool(name="csp", bufs=1)
        csp = cs_cm.__enter__()
        cos_a = csp.tile([128, NTF, HD], BF16, tag="cs", name="cs")
        sin_a = csp.tile([128, NTF, HD], BF16, tag="sn", name="sn")
        hT_cm = tc.tile_pool(name="hTp", bufs=1)
        hTp = hT_cm.__enter__()
        h_T = [hTp.tile([128, ND, 128], BF16, tag=f"hT{t}", name=f"hT{t}")
               for t in range(NTF)]

        def ap3(param, offset, s0, n0, s1, n1, s2, n2):
            return bass.AP(tensor=param.ap().tensor, offset=offset,
                           ap=[[s0, n0], [s1, n1], [s2, n2]])

        kqn_cm = tc.tile_pool(name="kqn", bufs=1)
        kqn = kqn_cm.__enter__()
        q_N = [kqn.tile([128, D], BF16, tag=f"qN{t}", bufs=1, name="qN")
               for t in range(NT)]

        def rope_norm(src_N, gname, out_ap, t, ropep):
            """src_N [128, D] bf16 -> norm+rope -> batched transpose."""
            rstd, mr = ln_stats(src_N)
            nrm = ropep.tile([128, D], BF16, tag="nrm", name="nrm")
            nc.vector.tensor_scalar(out=nrm, in0=src_N,
                                    scalar1=rstd, scalar2=mr,
                                    op0=ALU.mult, op1=ALU.subtract)
            if f"{gname}_g" in bc_tiles:
                nc.vector.tensor_mul(out=nrm, in0=nrm,
                                     in1=bc_tiles[f"{gname}_g"])
                nc.vector.tensor_add(out=nrm, in0=nrm,
                                     in1=bc_tiles[f"{gname}_b"])
            nrm3 = nrm.rearrange("p (h f) -> p h f", h=H)
            # rope with the half-swap folded into shifted-slice reads:
            # sw[.., 0:32] = nrm[.., 32:64]*sin[0:32]; sw[.., 32:64] =
            # nrm[.., 0:32]*sin[32:64]; out = nrm*cos + sw
            sw = ropep.tile([128, H, HD], BF16, tag="sw", name="sw")
            cos_bc = cos_a[:, t, :].unsqueeze(1).broadcast_to((128, H, HD))
            sb0 = sin_a[:, t, 0:32].unsqueeze(1).broadcast_to((128, H, 32))
            sb1 = sin_a[:, t, 32:64].unsqueeze(1).broadcast_to((128, H, 32))
            nc.vector.tensor_mul(out=sw[:, :, 0:32], in0=nrm3[:, :, 32:64],
                                 in1=sb0)
            nc.vector.tensor_mul(out=sw[:, :, 32:64], in0=nrm3[:, :, 0:32],
                                 in1=sb1)
            nc.vector.tensor_mul(out=nrm3, in0=nrm3, in1=cos_bc)
            nc.vector.tensor_add(out=nrm3, in0=nrm3, in1=sw)
            nc.scalar.dma_start_transpose(out=out_ap, in_=nrm)

        with (
            tc.tile_pool(name="xfp", bufs=1) as xfp,
            tc.tile_pool(name="wq", bufs=4) as wq,
            tc.tile_pool(name="mmps", bufs=2, space="PSUM") as mmps,
            tc.tile_pool(name="ropep", bufs=2) as ropep,
        ):
            def wq_load(col0):
                """[128, ND, 512] slice of wqkvT columns col0..col0+512."""
                w = wq.tile([128, ND, 512], BF16, tag="wqkv", name="wqkv")
                nc.sync.dma_start(
                    out=w, in_=ap3(wqkv_p, col0, D3, 128, 128 * D3, ND, 1, 512))
                return w

            # ---- Phase A/B/C interleaved: per 4-tile slab: LN1 +
            # transpose, then the dependent proj wave ---------------------
            def lnA(s, slabs):
                """LN1 + transpose for a 2-tile slab of xf."""
                xs = xfp.tile([128, 2, D], BF16, tag="xs", bufs=2, name="xs")
                nc.sync.dma_start(
                    out=xs, in_=ap3(xf_p, s * 2 * 128 * D, D, 128, 128 * D, 2, 1, D))
                slabs.append(xs)
                for i in range(2):
                    t = s * 2 + i
                    hb = xfp.tile([128, D], BF16, tag="hb", bufs=2, name="hb")
                    ln_normalize(xs[:, i, :], hb, "ln1")
                    nc.scalar.dma_start_transpose(out=h_T[t], in_=hb)

            def proj_group(w, tiles):
                ps = [mmps.tile([128, 512], F32, tag=f"ps{i}", name=f"ps{i}")
                      for i in range(len(tiles))]
                for i, t in enumerate(tiles):
                    for d in range(ND):
                        nc.tensor.matmul(
                            ps[i], h_T[t][:, d, :], w[:, d, :],
                            start=(d == 0), stop=(d == ND - 1))
                return ps

            def qk_write(dst, half, ps, bias_off, engine):
                sl = slice(half * 512, half * 512 + 512)
                if "b_qkv" in bc_tiles:
                    nc.vector.tensor_add(
                        out=dst[:, sl],
                        in0=bc_tiles["b_qkv"][:, bias_off:bias_off + 512],
                        in1=ps)
                elif engine == "v":
                    nc.vector.tensor_copy(out=dst[:, sl], in_=ps)
                else:
                    nc.scalar.activation(out=dst[:, sl], in_=ps, func=AF.Copy,
                                         bias=0.0, scale=1.0, alpha=0.0)

            slabs = []
            lnA(0, slabs)
            lnA(1, slabs)
            wqk = [wq_load(0), wq_load(512), wq_load(D), wq_load(D + 512)]
            # cos/sin: single DMAs; cos_a[p, t, j] = cosc[t*128+p, j]
            nc.sync.dma_start(
                out=cos_a, in_=ap3(cos_p, 0, HD, 128, 128 * HD, NTF, 1, HD))
            nc.sync.dma_start(
                out=sin_a, in_=ap3(sin_p, 0, HD, 128, 128 * HD, NTF, 1, HD))

            # Q proj (own tiles 0-3) then Q norm+rope
            for ch in (0, 1):
                ps = proj_group(wqk[ch], list(range(NT)))
                for i in range(NT):
                    qk_write(q_N[i], ch, ps[i], ch * 512, "v")
            for t in range(NT):
                rope_norm(q_N[t], "qn",
                          q_A[:, :, t * 128:(t + 1) * 128], t, ropep)
            nc.vector.tensor_copy(out=q_B[64:128, :, :], in_=q_A[64:128, :, :])
            nc.vector.memset(q_A[64:128, :, :], 0.0)
            nc.vector.memset(q_B[0:64, :, :], 0.0)
            nc.vector.memset(sel, 0.0)
            nc.vector.memset(sel[64:65, :], 1.0)
            for t in range(NTF):
                nc.vector.memset(v_pad[t][:, :, HD:HD + 1], 1.0)

            # K proj in waves of 4 tiles, norm+rope chasing per wave;
            # LN1 for the next slab interleaves with each wave
            for tg in range(NTF // 4):
                if tg + 1 < NTF // 4:
                    lnA(2 * (tg + 1), slabs)
                    lnA(2 * (tg + 1) + 1, slabs)
                tiles = list(range(tg * 4, tg * 4 + 4))
                k_N = [kqn.tile([128, D], BF16, tag="kN", bufs=4, name="kN")
                       for _ in tiles]
                for ci in (0, 1):
                    ps = proj_group(wqk[2 + ci], tiles)
                    for i in range(4):
                        qk_write(k_N[i], ci, ps[i], D + ci * 512,
                                 "s" if ci == 0 else "v")
                for i, t in enumerate(tiles):
                    rope_norm(k_N[i], "kn", k_T[t], t, ropep)

            # V proj
            wv = [wq_load(2 * D + ch * 512) for ch in (0, 1)]
            for tg in range(NTF // 4):
                tiles = list(range(tg * 4, tg * 4 + 4))
                for ci, w in enumerate(wv):
                    ps = proj_group(w, tiles)
                    h0 = ci * 8
                    for i, t in enumerate(tiles):
                        dst = v_pad[t][:, h0:h0 + 8, 0:HD]
                        if "b_qkv" in bc_tiles:
                            nc.vector.tensor_add(
                                out=dst,
                                in0=bc_tiles["b_qkv"][:, 2 * D + ci * 512:
                                                      2 * D + ci * 512 + 512]
                                .rearrange("p (h f) -> p h f", h=8),
                                in1=ps[i].rearrange("p (h f) -> p h f", h=8))
                        else:
                            nc.scalar.activation(
                                out=dst,
                                in_=ps[i].rearrange("p (h f) -> p h f", h=8),
                                func=AF.Copy, bias=0.0, scale=1.0, alpha=0.0)

        kqn_cm.__exit__(None, None, None)
        hT_cm.__exit__(None, None, None)
        cs_cm.__exit__(None, None, None)

        # ---- Phase E: attention -----------------------------------------
        attn_cm = tc.tile_pool(name="attnp", bufs=1)
        attnp = attn_cm.__enter__()
        attn_T = [attnp.tile([128, T], BF16, tag=f"at{d}", name=f"at{d}")
                  for d in range(ND)]
        scale = 1.0 / np.sqrt(HD)
        wo_cm = tc.tile_pool(name="wo", bufs=1)
        wo = wo_cm.__enter__()
        wob = wo.tile([128, ND, D], BF16, tag="wob", name="wob")
        nc.sync.dma_start(out=wob,
                          in_=ap3(wout_p, 0, D, 128, 128 * D, ND, 1, D))
        with (
            tc.tile_pool(name="scps", bufs=2, space="PSUM") as scps,
            tc.tile_pool(name="pvps", bufs=1, space="PSUM") as pvps,
            tc.tile_pool(name="bcps", bufs=1, space="PSUM") as bcps,
            tc.tile_pool(name="prb", bufs=24) as prb,
            tc.tile_pool(name="accp", bufs=2) as accp,
            tc.tile_pool(name="tbp", bufs=2) as tbp,
        ):
            prs = {}

            def emit_qk(d):
                """scores for head pair d: both heads in one 2-bank psum
                tile, one [128,1024] exp."""
                prs[d] = []
                for kt in range(NTF):
                    ps = scps.tile([128, 2, T], F32, tag="ps", name="ps")
                    nc.tensor.matmul(ps[:, 0, :], k_T[kt][:, d, :],
                                     q_A[:, d, :], start=True, stop=True)
                    nc.tensor.matmul(ps[:, 1, :], k_T[kt][:, d, :],
                                     q_B[:, d, :], start=True, stop=True)
                    pr = prb.tile([128, 2, T], BF16, tag="pr", name="pr")
                    nc.scalar.activation(out=pr, in_=ps, func=AF.Exp,
                                         scale=scale, alpha=0.0)
                    prs[d].append(pr)

            def emit_pv(d):
                pvA = pvps.tile([65, T], F32, tag="pvA", name="pvA")
                pvB = pvps.tile([65, T], F32, tag="pvB", name="pvB")
                hA, hB = 2 * d, 2 * d + 1
                for kt in range(NTF):
                    nc.tensor.matmul(pvA, v_pad[kt][:, hA, :],
                                     prs[d][kt][:, 0, :],
                                     start=(kt == 0), stop=(kt == NTF - 1))
                    nc.tensor.matmul(pvB, v_pad[kt][:, hB, :],
                                     prs[d][kt][:, 1, :],
                                     start=(kt == 0), stop=(kt == NTF - 1))
                accA = accp.tile([65, T], BF16, tag="accA", name="accA")
                accB = accp.tile([65, T], BF16, tag="accB", name="accB")
                nc.vector.tensor_copy(out=accA, in_=pvA)
                nc.vector.tensor_copy(out=accB, in_=pvB)
                bc = bcps.tile([128, 2, T], F32, tag="bc", name="bc")
                nc.tensor.matmul(bc[:, 0, :], sel, accA, start=True, stop=True)
                nc.tensor.matmul(bc[:, 1, :], sel, accB, start=True, stop=True)
                rc = accp.tile([128, 2, T], BF16, tag="rc", name="rc")
                with nc.allow_low_precision(reason="bf16 softmax denom"):
                    nc.vector.reciprocal(out=rc, in_=bc)
                nc.vector.tensor_mul(out=attn_T[d][0:64, :],
                                     in0=accA[0:64, :], in1=rc[0:64, 0, :])
                tmpB = tbp.tile([64, T], BF16, tag="tmpB", name="tmpB")
                nc.vector.tensor_mul(out=tmpB, in0=accB[0:64, :],
                                     in1=rc[0:64, 1, :])
                nc.sync.dma_start(out=attn_T[d][64:128, :], in_=tmpB)
                del prs[d]

            emit_qk(0)
            for d in range(HP):
                if d + 1 < HP:
                    emit_qk(d + 1)
                emit_pv(d)

        # ---- Phase F+G fused: out proj + residual + LN2 + transpose -----
        pr_cm = tc.tile_pool(name="prp", bufs=1)
        prp = pr_cm.__enter__()
        prod_T = [prp.tile([128, T], BF16, tag=f"pr{h}", name=f"pr{h}")
                  for h in range(NH)]
        h2_cm = tc.tile_pool(name="h2p", bufs=1)
        h2p = h2_cm.__enter__()
        h2_T = h2p.tile([128, ND, T], BF16, tag="h2T", name="h2T")
        with (
            tc.tile_pool(name="xop", bufs=1) as xop,
            tc.tile_pool(name="ops", bufs=2, space="PSUM") as ops,
            tc.tile_pool(name="h2w", bufs=2) as h2w,
        ):
            xo_t = xop.tile([128, NT, D], F32, tag="xo", name="xo")
            nc.sync.dma_start(out=xo_t,
                              in_=ap3(xo_p, 0, D, 128, 128 * D, NT, 1, D))
            for t in range(NT):
                ps = [ops.tile([128, 512], F32, tag=f"op{i}", name=f"op{i}")
                      for i in range(2)]
                for ch in range(2):
                    for d in range(ND):
                        nc.tensor.matmul(
                            ps[ch], attn_T[d][:, t * 128:(t + 1) * 128],
                            wob[:, d, ch * 512:(ch + 1) * 512],
                            start=(d == 0), stop=(d == ND - 1))
                for ch in range(2):
                    sl = slice(ch * 512, (ch + 1) * 512)
                    nc.vector.tensor_add(out=out1[t][:, sl],
                                         in0=xo_t[:, t, sl], in1=ps[ch])
                    if "b_out" in bc_tiles:
                        nc.vector.tensor_add(out=out1[t][:, sl],
                                             in0=out1[t][:, sl],
                                             in1=bc_tiles["b_out"][:, sl])
                h2 = h2w.tile([128, D], BF16, tag="h2", name="h2")
                ln_normalize(out1[t], h2, "ln2")
                nc.scalar.dma_start_transpose(
                    out=h2_T[:, :, t * 128:(t + 1) * 128], in_=h2)

        # ---- Phase H: FFN. FFN2's ch0 accumulation interleaves with FFN1
        # (4 psum banks each); ch1 runs as a dense second pass. ------------
        with (
            tc.tile_pool(name="wf", bufs=2) as wf,
            tc.tile_pool(name="w2p", bufs=2) as w2p,
            tc.tile_pool(name="ffps", bufs=2, space="PSUM") as ffps,
            tc.tile_pool(name="f2ps", bufs=1, space="PSUM") as f2ps,
            tc.tile_pool(name="s1p", bufs=2) as s1p,
            tc.tile_pool(name="finp", bufs=2) as finp,
        ):
            ps2 = [f2ps.tile([128, 512], F32, tag=f"f2{i}", name=f"f2{i}")
                   for i in range(NT)]

            def w2_load(hg, ch):
                w2b = w2p.tile([128, 4, 512], BF16, tag="w2b", name="w2b")
                nc.sync.dma_start(
                    out=w2b,
                    in_=ap3(w2_p, hg * 4 * 128 * D + ch * 512,
                            D, 128, 128 * D, 4, 1, 512))
                return w2b

            # FFN2-ch0 mms lag FFN1 by one ht so the PE never waits on the
            # vector mul that produces prod_T[ht]
            pending = []

            def flush_pending():
                for ht, w2b in pending:
                    for t in range(NT):
                        nc.tensor.matmul(
                            ps2[t], prod_T[ht][:, t * 128:(t + 1) * 128],
                            w2b[:, ht % 4, :],
                            start=(ht == 0), stop=(ht == NH - 1))
                pending.clear()

            for hg in range(NH // 4):
                w1b = wf.tile([128, ND, 512], BF16, tag="w1b", name="w1b")
                w3b = wf.tile([128, ND, 512], BF16, tag="w3b", name="w3b")
                nc.sync.dma_start(
                    out=w1b, in_=ap3(w1_p, hg * 512, FFN, 128, 128 * FFN, ND, 1, 512))
                nc.sync.dma_start(
                    out=w3b, in_=ap3(w3_p, hg * 512, FFN, 128, 128 * FFN, ND, 1, 512))
                w2b = w2_load(hg, 0)
                for i in range(4):
                    ht = hg * 4 + i
                    hsl = slice(i * 128, (i + 1) * 128)
                    ps1 = ffps.tile([128, T], F32, tag="ps1", name="ps1")
                    ps3 = ffps.tile([128, T], F32, tag="ps3", name="ps3")
                    for d in range(ND):
                        nc.tensor.matmul(ps1, w1b[:, d, hsl], h2_T[:, d, :],
                                         start=(d == 0), stop=(d == ND - 1))
                    flush_pending()
                    for d in range(ND):
                        nc.tensor.matmul(ps3, w3b[:, d, hsl], h2_T[:, d, :],
                                         start=(d == 0), stop=(d == ND - 1))
                    s1 = s1p.tile([128, T], BF16, tag="s1", name="s1")
                    b1arg = (bc_tiles["b1"][:, ht:ht + 1]
                             if "b1" in bc_tiles else 0.0)
                    nc.scalar.activation(out=s1, in_=ps1, func=AF.Silu,
                                         bias=b1arg, scale=1.0, alpha=0.0)
                    if "b3" in bc_tiles:
                        t3 = s1p.tile([128, T], F32, tag="t3", name="t3")
                        nc.vector.tensor_scalar_add(
                            out=t3, in0=ps3,
                            scalar1=bc_tiles["b3"][:, ht:ht + 1])
                        nc.vector.tensor_mul(out=prod_T[ht], in0=s1, in1=t3)
                    else:
                        nc.vector.tensor_mul(out=prod_T[ht], in0=s1, in1=ps3)
                    pending.append((ht, w2b))
            flush_pending()
            for t in range(NT):
                fin = finp.tile([128, 512], F32, tag="fin", name="fin")
                nc.vector.tensor_add(out=fin, in0=out1[t][:, 0:512],
                                     in1=ps2[t])
                if "b2" in bc_tiles:
                    nc.vector.tensor_add(out=fin, in0=fin,
                                         in1=bc_tiles["b2"][:, 0:512])
                nc.sync.dma_start(out=out_p.ap()[t * 128:(t + 1) * 128, 0:512],
                                  in_=fin)
            # ch1 second pass over stored prod_T (w2 ch1 halves re-loaded)
            psb = [ffps.tile([128, T], F32, tag="ps1", name="psb1"),
                   ffps.tile([128, T], F32, tag="ps3", name="psb3"),
                   ffps.tile([128, T], F32, tag="ps1", name="psb1b"),
                   ffps.tile([128, T], F32, tag="ps3", name="psb3b")]
            for hg in range(NH // 4):
                w2c = w2_load(hg, 1)
                for i in range(4):
                    ht = hg * 4 + i
                    for t in range(NT):
                        nc.tensor.matmul(
                            psb[t], prod_T[ht][:, t * 128:(t + 1) * 128],
                            w2c[:, i, :],
                            start=(ht == 0), stop=(ht == NH - 1))
            for t in range(NT):
                fin = finp.tile([128, 512], F32, tag="fin", name="fin")
                nc.vector.tensor_add(out=fin, in0=out1[t][:, 512:1024],
                                     in1=psb[t])
                if "b2" in bc_tiles:
                    nc.vector.tensor_add(out=fin, in0=fin,
                                         in1=bc_tiles["b2"][:, 512:1024])
                nc.sync.dma_start(
                    out=out_p.ap()[t * 128:(t + 1) * 128, 512:1024], in_=fin)

        h2_cm.__exit__(None, None, None)

        pr_cm.__exit__(None, None, None)
        wo_cm.__exit__(None, None, None)
        attn_cm.__exit__(None, None, None)
        kqv_cm.__exit__(None, None, None)
        o1_cm.__exit__(None, None, None)

    _split_all_waits(nc)
    return nc


# ---------------------------------------------------------------------------
# Host wrapper
# ---------------------------------------------------------------------------

_CACHE = {}
BF = ml_dtypes.bfloat16


def _prep_inputs(x, rope_cos, rope_sin, w_qkv, b_qkv, w_out, b_out,
                 qn_g, qn_b, kn_g, kn_b, ln1_g, ln1_b, ln2_g, ln2_b,
                 w1, b1, w2, b2, w3, b3):
    B, S, D = x.shape
    H, HD = 16, 64
    T = B * S // N_CORES

    flags = set()
    if not (np.all(ln1_g == 1) and np.all(ln1_b == 0)):
        flags.add("ln1_gb")
    if not (np.all(qn_g == 1) and np.all(qn_b == 0)):
        flags.add("qn_gb")
    if not (np.all(kn_g == 1) and np.all(kn_b == 0)):
        flags.add("kn_gb")
    if not (np.all(ln2_g == 1) and np.all(ln2_b == 0)):
        flags.add("ln2_gb")
    if np.any(b_qkv != 0):
        flags.add("bqkv")
    if np.any(b_out != 0):
        flags.add("bout")
    if np.any(b1 != 0):
        flags.add("b1")
    if np.any(b2 != 0):
        flags.add("b2")
    if np.any(b3 != 0):
        flags.add("b3")

    # compact rope tables with the rotation sign folded into sin
    sinmod = np.concatenate(
        [-rope_sin[:, :HD // 2], rope_sin[:, HD // 2:]], axis=1)  # [S, HD]
    cosc = np.asarray(rope_cos, np.float32)

    wqkvT = np.ascontiguousarray(w_qkv.T).astype(BF)
    woutT = np.ascontiguousarray(w_out.T).astype(BF)
    w1T = np.ascontiguousarray(w1.T).astype(BF)
    w3T = np.ascontiguousarray(w3.T).astype(BF)
    w2T = np.ascontiguousarray(w2.T).astype(BF)

    in_maps = []
    for c in range(N_CORES):
        b, qt = divmod(c, GROUP)
        o0 = qt * T
        perm = np.concatenate([np.arange(o0, o0 + T),
                               np.arange(0, o0),
                               np.arange(o0 + T, S)])
        xb = np.asarray(x[b], np.float32)
        m = {
            "xo": np.ascontiguousarray(xb[o0:o0 + T]),
            "xf": np.ascontiguousarray(xb[perm]).astype(BF),
            "cosc": np.ascontiguousarray(cosc[perm]).astype(BF),
            "sinc": np.ascontiguousarray(sinmod[perm]).astype(BF),
            "wqkvT": wqkvT, "woutT": woutT,
            "w1T": w1T, "w3T": w3T, "w2T": w2T,
        }
        opt = {"ln1_gb": [("ln1_g", ln1_g), ("ln1_b", ln1_b)],
               "qn_gb": [("qn_g", qn_g), ("qn_b", qn_b)],
               "kn_gb": [("kn_g", kn_g), ("kn_b", kn_b)],
               "ln2_gb": [("ln2_g", ln2_g), ("ln2_b", ln2_b)],
               "bqkv": [("b_qkv", b_qkv)], "bout": [("b_out", b_out)],
               "b1": [("b1", b1)], "b2": [("b2", b2)], "b3": [("b3", b3)]}
        for fl, items in opt.items():
            if fl in flags:
                for name, arr in items:
                    m[name] = np.ascontiguousarray(arr).astype(np.float32)
        in_maps.append(m)
    return in_maps, frozenset(flags), T, D


def kernel(**inputs):
    x = inputs["x"]
    B, S, D = x.shape
    in_maps, flags, T, _ = _prep_inputs(**inputs)

    key = (T, D, flags)
    if key not in _CACHE:
        _CACHE[key] = build_nc(T=T, D=D, flags=flags)
    nc = _CACHE[key]

    res = run_bass_kernel_spmd(nc, in_maps, core_ids=list(range(N_CORES)))
    out = np.empty((B * S, D), np.float32)
    for c in range(N_CORES):
        out[c * T:(c + 1) * T] = res.results[c]["out"]
    return out.reshape(B, S, D)


# revision 5
# speedup vs baseline: 1.0283x; 1.0005x over previous
"""Trainium2 Bass kernel v2: pre-norm attention + SwiGLU FFN layer.

Design: fully decoupled cores (NO collectives, no barrier). Tokens sharded
512/core (cores 0-3 batch 0, 4-7 batch 1); each core REDUNDANTLY computes
K,V for all 2048 tokens of its batch (+6.5 GFLOP) instead of all-gathering
them (-208us collective, no cross-core skew sensitivity). Per-core x is
reordered host-side so the core's own 512 tokens are always tiles 0-3
(attention is permutation-invariant over keys; rope tables are permuted
identically).

All matmuls bf16 (full PE rate, half SBUF/DMA), fp32 PSUM accumulation;
residual path fp32. All transposes are batched DMA-xbar transposes (one
instruction per [128,1024] tile) on the Activation HWDGE ring; all
weight/data loads are single big 3D-AP DMAs on the SP ring (per-DMA
dispatch costs ~0.6-1.2us serialized per ring, so instruction count on
each ring is minimized). The ones-column appended to V makes PV emit
softmax denominators for free; softmax skips max subtraction (scores O(1)
after QK-norm). bf16+tile_position crashes this HW (probe-verified), so
per-head K=64 QK contractions run as plain K=128 matmuls against
zero-masked q copies (qA/qB).
"""

import numpy as np
import ml_dtypes

import bass_rust
import concourse.bass as bass
import concourse.mybir as mybir
import concourse.tile as tile
from concourse.bass_utils import run_bass_kernel_spmd
from concourse.vector_clock import ScopedClock

F32 = mybir.dt.float32
BF16 = mybir.dt.bfloat16
AF = mybir.ActivationFunctionType
ALU = mybir.AluOpType

N_CORES = 8
GROUP = 4
EPS = 1e-6

# ---------------------------------------------------------------------------
# Workaround for this walrus build's 1-wait-per-instruction encoding limit.
# ---------------------------------------------------------------------------
_MAX_WAITS = 1
_carrier_id = [0]


def _patched_drain_and_barrier(self, tick_clock, wait_clock):
    nc = self.nc
    drain_inst = nc.sync.drain()
    wait_clock.add_sem_waits(
        drain_inst.ins, ScopedClock({None: tick_clock.global_clock})
    )
    si = drain_inst.ins.sync_info
    waits = list(si.on_wait)
    if len(waits) > _MAX_WAITS:
        drain_inst.ins.sync_info = bass_rust.SyncInfo(
            on_wait=waits[:_MAX_WAITS], on_update=list(si.on_update)
        )
        rest = waits[_MAX_WAITS:]
        while rest:
            chunk, rest = rest[:_MAX_WAITS], rest[_MAX_WAITS:]
            extra = nc.sync.drain()
            extra.ins.sync_info = bass_rust.SyncInfo(on_wait=chunk, on_update=[])

    nc.all_engine_barrier()
    assert self.sems is not None
    popped = nc._tile_sem_poison_stack.pop()
    assert popped is self._sem_poison
    nc.clear_and_free_semaphores(list(self.sems.allocated().values()))
    nc.all_engine_barrier()


tile.TileContext._drain_and_barrier = _patched_drain_and_barrier


def _split_all_waits(nc, max_waits=_MAX_WAITS):
    for fn in nc.m.functions:
        for bb in fn.blocks:
            insts = list(bb.instructions)
            out = []
            changed = False
            for inst in insts:
                si = getattr(inst, "sync_info", None)
                if si is not None and si.on_wait and len(si.on_wait) > max_waits:
                    waits = list(si.on_wait)
                    updates = list(si.on_update)
                    extra, keep = waits[:-max_waits], waits[-max_waits:]
                    while extra:
                        chunk, extra = extra[:max_waits], extra[max_waits:]
                        _carrier_id[0] += 1
                        nop = mybir.InstNoOp(name=f"I-waitcar-{_carrier_id[0]}")
                        nop.engine = inst.engine
                        nop.sync_info = bass_rust.SyncInfo(on_wait=chunk, on_update=[])
                        nc.register_instruction(nop)
                        out.append(nop)
                    inst.sync_info = bass_rust.SyncInfo(on_wait=keep, on_update=updates)
                    changed = True
                out.append(inst)
            if changed:
                bb.instructions = out


# ---------------------------------------------------------------------------
# Graph builder (one SPMD program; cores fully independent)
# ---------------------------------------------------------------------------

def build_nc(T=512, D=1024, H=16, HD=64, FFN=4096, flags=frozenset()):
    """T: own tokens per core; TF = GROUP*T batch context."""
    TF = GROUP * T           # full-batch tokens (2048)
    NT = T // 128            # own token tiles (4)
    NTF = TF // 128          # full token tiles (16)
    ND = D // 128            # model-dim tiles (8)
    NH = FFN // 128          # ffn hidden tiles (32)
    HP = H // 2              # head pairs (8)
    D3 = 3 * D

    nc = bass.Bass(trn_type="TRN2", num_devices=N_CORES)

    xo_p = nc.declare_dram_parameter("xo", [T, D], F32, isOutput=False)
    xf_p = nc.declare_dram_parameter("xf", [TF, D], BF16, isOutput=False)
    cos_p = nc.declare_dram_parameter("cosc", [TF, HD], BF16, isOutput=False)
    sin_p = nc.declare_dram_parameter("sinc", [TF, HD], BF16, isOutput=False)
    wqkv_p = nc.declare_dram_parameter("wqkvT", [D, D3], BF16, isOutput=False)
    wout_p = nc.declare_dram_parameter("woutT", [D, D], BF16, isOutput=False)
    w1_p = nc.declare_dram_parameter("w1T", [D, FFN], BF16, isOutput=False)
    w3_p = nc.declare_dram_parameter("w3T", [D, FFN], BF16, isOutput=False)
    w2_p = nc.declare_dram_parameter("w2T", [FFN, D], BF16, isOutput=False)
    vecs = {}
    for name, size in [("ln1_g", D), ("ln1_b", D), ("qn_g", D), ("qn_b", D),
                       ("kn_g", D), ("kn_b", D), ("ln2_g", D), ("ln2_b", D),
                       ("b_qkv", D3), ("b_out", D), ("b1", FFN), ("b3", FFN),
                       ("b2", D)]:
        flag = {"ln1_g": "ln1_gb", "ln1_b": "ln1_gb", "qn_g": "qn_gb",
                "qn_b": "qn_gb", "kn_g": "kn_gb", "kn_b": "kn_gb",
                "ln2_g": "ln2_gb", "ln2_b": "ln2_gb", "b_qkv": "bqkv",
                "b_out": "bout", "b1": "b1", "b3": "b3", "b2": "b2"}[name]
        if flag in flags:
            vecs[name] = nc.declare_dram_parameter(name, [size], F32, isOutput=False)
    out_p = nc.declare_dram_parameter("out", [T, D], F32, isOutput=True)

    def bcast_ap(param, width):
        return bass.AP(tensor=param.ap().tensor, offset=0,
                       ap=[[0, 128], [1, width]])

    from contextlib import ExitStack
    with tile.TileContext(nc) as tc, ExitStack() as stack:
        const = stack.enter_context(tc.tile_pool(name="const", bufs=1))
        sel = const.tile([65, 128], BF16, tag="sel")
        eps_t = const.tile([128, 1], F32, tag="eps")
        nc.vector.memset(eps_t, EPS)

        bc_tiles = {}
        for name in ("ln1_g", "ln1_b", "qn_g", "qn_b", "kn_g", "kn_b",
                     "ln2_g", "ln2_b", "b_out", "b2"):
            if name in vecs:
                t = const.tile([128, D], F32, tag=f"bc_{name}", name=f"bc_{name}")
                nc.sync.dma_start(out=t, in_=bcast_ap(vecs[name], D))
                bc_tiles[name] = t
        if "b_qkv" in vecs:
            t = const.tile([128, D3], F32, tag="bc_bqkv")
            nc.sync.dma_start(out=t, in_=bcast_ap(vecs["b_qkv"], D3))
            bc_tiles["b_qkv"] = t
        for name in ("b1", "b3"):
            if name in vecs:
                t = const.tile([128, NH], F32, tag=f"col_{name}", name=f"col_{name}")
                ap = bass.AP(tensor=vecs[name].ap().tensor, offset=0,
                             ap=[[1, 128], [128, NH]])
                nc.sync.dma_start(out=t, in_=ap)
                bc_tiles[name] = t

        stat = stack.enter_context(tc.tile_pool(name="stat", bufs=4))

        def ln_stats(src_tile):
            """mean/rstd of a [128, D] tile -> (rstd, negmr) [128,1] f32."""
            st = stat.tile([128, 2, 6], F32, tag="lnst", name="lnst")
            nc.vector.bn_stats(out=st[:, 0, :], in_=src_tile[:, 0:512])
            nc.vector.bn_stats(out=st[:, 1, :], in_=src_tile[:, 512:1024])
            mv = stat.tile([128, 2], F32, tag="lnmv", name="lnmv")
            nc.vector.bn_aggr(out=mv, in_=st)
            rstd = stat.tile([128, 1], F32, tag="lnrstd", name="lnrstd")
            nc.scalar.activation(out=rstd, in_=mv[:, 1:2], func=AF.Sqrt,
                                 bias=eps_t, scale=1.0, alpha=0.0)
            nc.vector.reciprocal(out=rstd, in_=rstd)
            mr = stat.tile([128, 1], F32, tag="lnmr", name="lnmr")
            nc.vector.tensor_mul(out=mr, in0=mv[:, 0:1], in1=rstd)
            return rstd, mr

        def ln_normalize(src_tile, dst_tile, gname):
            """LN via vector tensor_scalar: dst = src*rstd - mean*rstd."""
            rstd, mr = ln_stats(src_tile)
            nc.vector.tensor_scalar(out=dst_tile, in0=src_tile,
                                    scalar1=rstd, scalar2=mr,
                                    op0=ALU.mult, op1=ALU.subtract)
            if f"{gname}_g" in bc_tiles:
                nc.vector.tensor_mul(out=dst_tile, in0=dst_tile,
                                     in1=bc_tiles[f"{gname}_g"])
                nc.vector.tensor_add(out=dst_tile, in0=dst_tile,
                                     in1=bc_tiles[f"{gname}_b"])

        # ---- persistent tiles (entered in LIFO order by lifetime end) ---
        o1_cm = tc.tile_pool(name="o1p", bufs=1)
        o1p = o1_cm.__enter__()
        out1 = [o1p.tile([128, D], F32, tag=f"o1{t}", name=f"o1{t}")
                for t in range(NT)]
        kqv_cm = tc.tile_pool(name="kqvp", bufs=1)
        kqvp = kqv_cm.__enter__()
        # k_T split per token tile so QK(kt) depends only on its own
        # transpose (a single shared tile serializes readers on ALL writers)
        k_T = [kqvp.tile([128, ND, 128], BF16, tag=f"kT{t}", name=f"kT{t}")
               for t in range(NTF)]
        q_A = kqvp.tile([128, ND, T], BF16, tag="qA", name="qA")
        q_B = kqvp.tile([128, ND, T], BF16, tag="qB", name="qB")
        v_pad = [kqvp.tile([128, H, HD + 1], BF16, tag=f"vp{t}", name=f"vp{t}")
                 for t in range(NTF)]
        cs_cm = tc.tile_pool(name="csp", bufs=1)
        csp = cs_cm.__enter__()
        cos_a = csp.tile([128, NTF, HD], BF16, tag="cs", name="cs")
        sin_a = csp.tile([128, NTF, HD], BF16, tag="sn", name="sn")
        hT_cm = tc.tile_pool(name="hTp", bufs=1)
        hTp = hT_cm.__enter__()
        h_T = [hTp.tile([128, ND, 128], BF16, tag=f"hT{t}", name=f"hT{t}")
               for t in range(NTF)]

        def ap3(param, offset, s0, n0, s1, n1, s2, n2):
            return bass.AP(tensor=param.ap().tensor, offset=offset,
                           ap=[[s0, n0], [s1, n1], [s2, n2]])

        kqn_cm = tc.tile_pool(name="kqn", bufs=1)
        kqn = kqn_cm.__enter__()
        q_N = [kqn.tile([128, D], BF16, tag=f"qN{t}", bufs=1, name="qN")
               for t in range(NT)]

        def rope_norm(src_N, gname, out_ap, t, ropep):
            """src_N [128, D] bf16 -> norm+rope -> batched transpose."""
            rstd, mr = ln_stats(src_N)
            nrm = ropep.tile([128, D], BF16, tag="nrm", name="nrm")
            nc.vector.tensor_scalar(out=nrm, in0=src_N,
                                    scalar1=rstd, scalar2=mr,
                                    op0=ALU.mult, op1=ALU.subtract)
            if f"{gname}_g" in bc_tiles:
                nc.vector.tensor_mul(out=nrm, in0=nrm,
                                     in1=bc_tiles[f"{gname}_g"])
                nc.vector.tensor_add(out=nrm, in0=nrm,
                                     in1=bc_tiles[f"{gname}_b"])
            nrm3 = nrm.rearrange("p (h f) -> p h f", h=H)
            # rope with the half-swap folded into shifted-slice reads:
            # sw[.., 0:32] = nrm[.., 32:64]*sin[0:32]; sw[.., 32:64] =
            # nrm[.., 0:32]*sin[32:64]; out = nrm*cos + sw
            sw = ropep.tile([128, H, HD], BF16, tag="sw", name="sw")
            cos_bc = cos_a[:, t, :].unsqueeze(1).broadcast_to((128, H, HD))
            sb0 = sin_a[:, t, 0:32].unsqueeze(1).broadcast_to((128, H, 32))
            sb1 = sin_a[:, t, 32:64].unsqueeze(1).broadcast_to((128, H, 32))
            nc.vector.tensor_mul(out=sw[:, :, 0:32], in0=nrm3[:, :, 32:64],
                                 in1=sb0)
            nc.vector.tensor_mul(out=sw[:, :, 32:64], in0=nrm3[:, :, 0:32],
                                 in1=sb1)
            nc.vector.tensor_mul(out=nrm3, in0=nrm3, in1=cos_bc)
            nc.vector.tensor_add(out=nrm3, in0=nrm3, in1=sw)
            nc.scalar.dma_start_transpose(out=out_ap, in_=nrm)

        with (
            tc.tile_pool(name="xfp", bufs=1) as xfp,
            tc.tile_pool(name="wq", bufs=4) as wq,
            tc.tile_pool(name="mmps", bufs=2, space="PSUM") as mmps,
            tc.tile_pool(name="ropep", bufs=2) as ropep,
        ):
            def wq_load(col0):
                """[128, ND, 512] slice of wqkvT columns col0..col0+512."""
                w = wq.tile([128, ND, 512], BF16, tag="wqkv", name="wqkv")
                nc.sync.dma_start(
                    out=w, in_=ap3(wqkv_p, col0, D3, 128, 128 * D3, ND, 1, 512))
                return w

            # ---- Phase A/B/C interleaved: per 4-tile slab: LN1 +
            # transpose, then the dependent proj wave ---------------------
            def lnA(s, slabs):
                """LN1 + transpose for a 2-tile slab of xf."""
                xs = xfp.tile([128, 2, D], BF16, tag="xs", bufs=2, name="xs")
                nc.sync.dma_start(
                    out=xs, in_=ap3(xf_p, s * 2 * 128 * D, D, 128, 128 * D, 2, 1, D))
                slabs.append(xs)
                for i in range(2):
                    t = s * 2 + i
                    hb = xfp.tile([128, D], BF16, tag="hb", bufs=2, name="hb")
                    ln_normalize(xs[:, i, :], hb, "ln1")
                    nc.scalar.dma_start_transpose(out=h_T[t], in_=hb)

            def proj_group(w, tiles):
                ps = [mmps.tile([128, 512], F32, tag=f"ps{i}", name=f"ps{i}")
                      for i in range(len(tiles))]
                for i, t in enumerate(tiles):
                    for d in range(ND):
                        nc.tensor.matmul(
                            ps[i], h_T[t][:, d, :], w[:, d, :],
                            start=(d == 0), stop=(d == ND - 1))
                return ps

            def qk_write(dst, half, ps, bias_off, engine):
                sl = slice(half * 512, half * 512 + 512)
                if "b_qkv" in bc_tiles:
                    nc.vector.tensor_add(
                        out=dst[:, sl],
                        in0=bc_tiles["b_qkv"][:, bias_off:bias_off + 512],
                        in1=ps)
                elif engine == "v":
                    nc.vector.tensor_copy(out=dst[:, sl], in_=ps)
                else:
                    nc.scalar.activation(out=dst[:, sl], in_=ps, func=AF.Copy,
                                         bias=0.0, scale=1.0, alpha=0.0)

            slabs = []
            lnA(0, slabs)
            lnA(1, slabs)
            wqk = [wq_load(0), wq_load(512), wq_load(D), wq_load(D + 512)]
            # cos/sin: single DMAs; cos_a[p, t, j] = cosc[t*128+p, j]
            nc.sync.dma_start(
                out=cos_a, in_=ap3(cos_p, 0, HD, 128, 128 * HD, NTF, 1, HD))
            nc.sync.dma_start(
                out=sin_a, in_=ap3(sin_p, 0, HD, 128, 128 * HD, NTF, 1, HD))

            # Q proj (own tiles 0-3) then Q norm+rope
            for ch in (0, 1):
                ps = proj_group(wqk[ch], list(range(NT)))
                for i in range(NT):
                    qk_write(q_N[i], ch, ps[i], ch * 512, "v")
            for t in range(NT):
                rope_norm(q_N[t], "qn",
                          q_A[:, :, t * 128:(t + 1) * 128], t, ropep)
            nc.vector.tensor_copy(out=q_B[64:128, :, :], in_=q_A[64:128, :, :])
            nc.vector.memset(q_A[64:128, :, :], 0.0)
            nc.vector.memset(q_B[0:64, :, :], 0.0)
            nc.vector.memset(sel, 0.0)
            nc.vector.memset(sel[64:65, :], 1.0)
            for t in range(NTF):
                nc.vector.memset(v_pad[t][:, :, HD:HD + 1], 1.0)

            # K proj in waves of 4 tiles, norm+rope chasing per wave;
            # LN1 for the next slab interleaves with each wave
            for tg in range(NTF // 4):
                if tg + 1 < NTF // 4:
                    lnA(2 * (tg + 1), slabs)
                    lnA(2 * (tg + 1) + 1, slabs)
                tiles = list(range(tg * 4, tg * 4 + 4))
                k_N = [kqn.tile([128, D], BF16, tag="kN", bufs=4, name="kN")
                       for _ in tiles]
                for ci in (0, 1):
                    ps = proj_group(wqk[2 + ci], tiles)
                    for i in range(4):
                        qk_write(k_N[i], ci, ps[i], D + ci * 512, "s")
                for i, t in enumerate(tiles):
                    rope_norm(k_N[i], "kn", k_T[t], t, ropep)

            # V proj
            wv = [wq_load(2 * D + ch * 512) for ch in (0, 1)]
            for tg in range(NTF // 4):
                tiles = list(range(tg * 4, tg * 4 + 4))
                for ci, w in enumerate(wv):
                    ps = proj_group(w, tiles)
                    h0 = ci * 8
                    for i, t in enumerate(tiles):
                        dst = v_pad[t][:, h0:h0 + 8, 0:HD]
                        if "b_qkv" in bc_tiles:
                            nc.vector.tensor_add(
                                out=dst,
                                in0=bc_tiles["b_qkv"][:, 2 * D + ci * 512:
                                                      2 * D + ci * 512 + 512]
                                .rearrange("p (h f) -> p h f", h=8),
                                in1=ps[i].rearrange("p (h f) -> p h f", h=8))
                        else:
                            nc.scalar.activation(
                                out=dst,
                                in_=ps[i].rearrange("p (h f) -> p h f", h=8),
                                func=AF.Copy, bias=0.0, scale=1.0, alpha=0.0)

        kqn_cm.__exit__(None, None, None)
        hT_cm.__exit__(None, None, None)
        cs_cm.__exit__(None, None, None)

        # ---- Phase E: attention -----------------------------------------
        attn_cm = tc.tile_pool(name="attnp", bufs=1)
        attnp = attn_cm.__enter__()
        attn_T = [attnp.tile([128, T], BF16, tag=f"at{d}", name=f"at{d}")
                  for d in range(ND)]
        scale = 1.0 / np.sqrt(HD)
        wo_cm = tc.tile_pool(name="wo", bufs=1)
        wo = wo_cm.__enter__()
        wob = wo.tile([128, ND, D], BF16, tag="wob", name="wob")
        nc.sync.dma_start(out=wob,
                          in_=ap3(wout_p, 0, D, 128, 128 * D, ND, 1, D))
        with (
            tc.tile_pool(name="scps", bufs=2, space="PSUM") as scps,
            tc.tile_pool(name="pvps", bufs=1, space="PSUM") as pvps,
            tc.tile_pool(name="bcps", bufs=1, space="PSUM") as bcps,
            tc.tile_pool(name="prb", bufs=24) as prb,
            tc.tile_pool(name="accp", bufs=2) as accp,
            tc.tile_pool(name="tbp", bufs=2) as tbp,
        ):
            prs = {}

            def emit_qk(d):
                """scores for head pair d: both heads in one 2-bank psum
                tile, one [128,1024] exp."""
                prs[d] = []
                for kt in range(NTF):
                    ps = scps.tile([128, 2, T], F32, tag="ps", name="ps")
                    nc.tensor.matmul(ps[:, 0, :], k_T[kt][:, d, :],
                                     q_A[:, d, :], start=True, stop=True)
                    nc.tensor.matmul(ps[:, 1, :], k_T[kt][:, d, :],
                                     q_B[:, d, :], start=True, stop=True)
                    pr = prb.tile([128, 2, T], BF16, tag="pr", name="pr")
                    nc.scalar.activation(out=pr, in_=ps, func=AF.Exp,
                                         scale=scale, alpha=0.0)
                    prs[d].append(pr)

            def emit_pv(d):
                pvA = pvps.tile([65, T], F32, tag="pvA", name="pvA")
                pvB = pvps.tile([65, T], F32, tag="pvB", name="pvB")
                hA, hB = 2 * d, 2 * d + 1
                for kt in range(NTF):
                    nc.tensor.matmul(pvA, v_pad[kt][:, hA, :],
                                     prs[d][kt][:, 0, :],
                                     start=(kt == 0), stop=(kt == NTF - 1))
                    nc.tensor.matmul(pvB, v_pad[kt][:, hB, :],
                                     prs[d][kt][:, 1, :],
                                     start=(kt == 0), stop=(kt == NTF - 1))
                accA = accp.tile([65, T], BF16, tag="accA", name="accA")
                accB = accp.tile([65, T], BF16, tag="accB", name="accB")
                nc.vector.tensor_copy(out=accA, in_=pvA)
                nc.vector.tensor_copy(out=accB, in_=pvB)
                bc = bcps.tile([128, 2, T], F32, tag="bc", name="bc")
                nc.tensor.matmul(bc[:, 0, :], sel, accA, start=True, stop=True)
                nc.tensor.matmul(bc[:, 1, :], sel, accB, start=True, stop=True)
                rc = accp.tile([128, 2, T], BF16, tag="rc", name="rc")
                with nc.allow_low_precision(reason="bf16 softmax denom"):
                    nc.vector.reciprocal(out=rc, in_=bc)
                nc.vector.tensor_mul(out=attn_T[d][0:64, :],
                                     in0=accA[0:64, :], in1=rc[0:64, 0, :])
                tmpB = tbp.tile([64, T], BF16, tag="tmpB", name="tmpB")
                nc.vector.tensor_mul(out=tmpB, in0=accB[0:64, :],
                                     in1=rc[0:64, 1, :])
                nc.sync.dma_start(out=attn_T[d][64:128, :], in_=tmpB)
                del prs[d]

            emit_qk(0)
            for d in range(HP):
                if d + 1 < HP:
                    emit_qk(d + 1)
                emit_pv(d)

        # ---- Phase F+G fused: out proj + residual + LN2 + transpose -----
        pr_cm = tc.tile_pool(name="prp", bufs=1)
        prp = pr_cm.__enter__()
        prod_T = [prp.tile([128, T], BF16, tag=f"pr{h}", name=f"pr{h}")
                  for h in range(NH)]
        h2_cm = tc.tile_pool(name="h2p", bufs=1)
        h2p = h2_cm.__enter__()
        h2_T = h2p.tile([128, ND, T], BF16, tag="h2T", name="h2T")
        with (
            tc.tile_pool(name="xop", bufs=1) as xop,
            tc.tile_pool(name="ops", bufs=2, space="PSUM") as ops,
            tc.tile_pool(name="h2w", bufs=2) as h2w,
        ):
            xo_t = xop.tile([128, NT, D], F32, tag="xo", name="xo")
            nc.sync.dma_start(out=xo_t,
                              in_=ap3(xo_p, 0, D, 128, 128 * D, NT, 1, D))
            for t in range(NT):
                ps = [ops.tile([128, 512], F32, tag=f"op{i}", name=f"op{i}")
                      for i in range(2)]
                for ch in range(2):
                    for d in range(ND):
                        nc.tensor.matmul(
                            ps[ch], attn_T[d][:, t * 128:(t + 1) * 128],
                            wob[:, d, ch * 512:(ch + 1) * 512],
                            start=(d == 0), stop=(d == ND - 1))
                for ch in range(2):
                    sl = slice(ch * 512, (ch + 1) * 512)
                    nc.vector.tensor_add(out=out1[t][:, sl],
                                         in0=xo_t[:, t, sl], in1=ps[ch])
                    if "b_out" in bc_tiles:
                        nc.vector.tensor_add(out=out1[t][:, sl],
                                             in0=out1[t][:, sl],
                                             in1=bc_tiles["b_out"][:, sl])
                h2 = h2w.tile([128, D], BF16, tag="h2", name="h2")
                ln_normalize(out1[t], h2, "ln2")
                nc.scalar.dma_start_transpose(
                    out=h2_T[:, :, t * 128:(t + 1) * 128], in_=h2)

        # ---- Phase H: FFN. FFN2's ch0 accumulation interleaves with FFN1
        # (4 psum banks each); ch1 runs as a dense second pass. ------------
        with (
            tc.tile_pool(name="wf", bufs=2) as wf,
            tc.tile_pool(name="w2p", bufs=2) as w2p,
            tc.tile_pool(name="ffps", bufs=2, space="PSUM") as ffps,
            tc.tile_pool(name="f2ps", bufs=1, space="PSUM") as f2ps,
            tc.tile_pool(name="s1p", bufs=2) as s1p,
            tc.tile_pool(name="finp", bufs=2) as finp,
        ):
            ps2 = [f2ps.tile([128, 512], F32, tag=f"f2{i}", name=f"f2{i}")
                   for i in range(NT)]

            def w2_load(hg, ch):
                w2b = w2p.tile([128, 4, 512], BF16, tag="w2b", name="w2b")
                nc.sync.dma_start(
                    out=w2b,
                    in_=ap3(w2_p, hg * 4 * 128 * D + ch * 512,
                            D, 128, 128 * D, 4, 1, 512))
                return w2b

            # FFN2-ch0 mms lag FFN1 by one ht so the PE never waits on the
            # vector mul that produces prod_T[ht]
            pending = []

            def flush_pending():
                for ht, w2b in pending:
                    for t in range(NT):
                        nc.tensor.matmul(
                            ps2[t], prod_T[ht][:, t * 128:(t + 1) * 128],
                            w2b[:, ht % 4, :],
                            start=(ht == 0), stop=(ht == NH - 1))
                pending.clear()

            for hg in range(NH // 4):
                w1b = wf.tile([128, ND, 512], BF16, tag="w1b", name="w1b")
                w3b = wf.tile([128, ND, 512], BF16, tag="w3b", name="w3b")
                nc.sync.dma_start(
                    out=w1b, in_=ap3(w1_p, hg * 512, FFN, 128, 128 * FFN, ND, 1, 512))
                nc.sync.dma_start(
                    out=w3b, in_=ap3(w3_p, hg * 512, FFN, 128, 128 * FFN, ND, 1, 512))
                w2b = w2_load(hg, 0)
                for i in range(4):
                    ht = hg * 4 + i
                    hsl = slice(i * 128, (i + 1) * 128)
                    ps1 = ffps.tile([128, T], F32, tag="ps1", name="ps1")
                    ps3 = ffps.tile([128, T], F32, tag="ps3", name="ps3")
                    for d in range(ND):
                        nc.tensor.matmul(ps1, w1b[:, d, hsl], h2_T[:, d, :],
                                         start=(d == 0), stop=(d == ND - 1))
                    flush_pending()
                    for d in range(ND):
                        nc.tensor.matmul(ps3, w3b[:, d, hsl], h2_T[:, d, :],
                                         start=(d == 0), stop=(d == ND - 1))
                    s1 = s1p.tile([128, T], BF16, tag="s1", name="s1")
                    b1arg = (bc_tiles["b1"][:, ht:ht + 1]
                             if "b1" in bc_tiles else 0.0)
                    nc.scalar.activation(out=s1, in_=ps1, func=AF.Silu,
                                         bias=b1arg, scale=1.0, alpha=0.0)
                    if "b3" in bc_tiles:
                        t3 = s1p.tile([128, T], F32, tag="t3", name="t3")
                        nc.vector.tensor_scalar_add(
                            out=t3, in0=ps3,
                            scalar1=bc_tiles["b3"][:, ht:ht + 1])
                        nc.vector.tensor_mul(out=prod_T[ht], in0=s1, in1=t3)
                    else:
                        nc.vector.tensor_mul(out=prod_T[ht], in0=s1, in1=ps3)
                    pending.append((ht, w2b))
            flush_pending()
            for t in range(NT):
                fin = finp.tile([128, 512], F32, tag="fin", name="fin")
                nc.vector.tensor_add(out=fin, in0=out1[t][:, 0:512],
                                     in1=ps2[t])
                if "b2" in bc_tiles:
                    nc.vector.tensor_add(out=fin, in0=fin,
                                         in1=bc_tiles["b2"][:, 0:512])
                nc.sync.dma_start(out=out_p.ap()[t * 128:(t + 1) * 128, 0:512],
                                  in_=fin)
            # ch1 second pass over stored prod_T (w2 ch1 halves re-loaded)
            psb = [ffps.tile([128, T], F32, tag="ps1", name="psb1"),
                   ffps.tile([128, T], F32, tag="ps3", name="psb3"),
                   ffps.tile([128, T], F32, tag="ps1", name="psb1b"),
                   ffps.tile([128, T], F32, tag="ps3", name="psb3b")]
            for hg in range(NH // 4):
                w2c = w2_load(hg, 1)
                for i in range(4):
                    ht = hg * 4 + i
                    for t in range(NT):
                        nc.tensor.matmul(
                            psb[t], prod_T[ht][:, t * 128:(t + 1) * 128],
                            w2c[:, i, :],
                            start=(ht == 0), stop=(ht == NH - 1))
            for t in range(NT):
                fin = finp.tile([128, 512], F32, tag="fin", name="fin")
                nc.vector.tensor_add(out=fin, in0=out1[t][:, 512:1024],
                                     in1=psb[t])
                if "b2" in bc_tiles:
                    nc.vector.tensor_add(out=fin, in0=fin,
                                         in1=bc_tiles["b2"][:, 512:1024])
                nc.sync.dma_start(
                    out=out_p.ap()[t * 128:(t + 1) * 128, 512:1024], in_=fin)

        h2_cm.__exit__(None, None, None)

        pr_cm.__exit__(None, None, None)
        wo_cm.__exit__(None, None, None)
        attn_cm.__exit__(None, None, None)
        kqv_cm.__exit__(None, None, None)
        o1_cm.__exit__(None, None, None)

    _split_all_waits(nc)
    return nc


# ---------------------------------------------------------------------------
# Host wrapper
# ---------------------------------------------------------------------------

_CACHE = {}
BF = ml_dtypes.bfloat16


def _prep_inputs(x, rope_cos, rope_sin, w_qkv, b_qkv, w_out, b_out,
                 qn_g, qn_b, kn_g, kn_b, ln1_g, ln1_b, ln2_g, ln2_b,
                 w1, b1, w2, b2, w3, b3):
    B, S, D = x.shape
    H, HD = 16, 64
    T = B * S // N_CORES

    flags = set()
    if not (np.all(ln1_g == 1) and np.all(ln1_b == 0)):
        flags.add("ln1_gb")
    if not (np.all(qn_g == 1) and np.all(qn_b == 0)):
        flags.add("qn_gb")
    if not (np.all(kn_g == 1) and np.all(kn_b == 0)):
        flags.add("kn_gb")
    if not (np.all(ln2_g == 1) and np.all(ln2_b == 0)):
        flags.add("ln2_gb")
    if np.any(b_qkv != 0):
        flags.add("bqkv")
    if np.any(b_out != 0):
        flags.add("bout")
    if np.any(b1 != 0):
        flags.add("b1")
    if np.any(b2 != 0):
        flags.add("b2")
    if np.any(b3 != 0):
        flags.add("b3")

    # compact rope tables with the rotation sign folded into sin
    sinmod = np.concatenate(
        [-rope_sin[:, :HD // 2], rope_sin[:, HD // 2:]], axis=1)  # [S, HD]
    cosc = np.asarray(rope_cos, np.float32)

    wqkvT = np.ascontiguousarray(w_qkv.T).astype(BF)
    woutT = np.ascontiguousarray(w_out.T).astype(BF)
    w1T = np.ascontiguousarray(w1.T).astype(BF)
    w3T = np.ascontiguousarray(w3.T).astype(BF)
    w2T = np.ascontiguousarray(w2.T).astype(BF)

    in_maps = []
    for c in range(N_CORES):
        b, qt = divmod(c, GROUP)
        o0 = qt * T
        perm = np.concatenate([np.arange(o0, o0 + T),
                               np.arange(0, o0),
                               np.arange(o0 + T, S)])
        xb = np.asarray(x[b], np.float32)
        m = {
            "xo": np.ascontiguousarray(xb[o0:o0 + T]),
            "xf": np.ascontiguousarray(xb[perm]).astype(BF),
            "cosc": np.ascontiguousarray(cosc[perm]).astype(BF),
            "sinc": np.ascontiguousarray(sinmod[perm]).astype(BF),
            "wqkvT": wqkvT, "woutT": woutT,
            "w1T": w1T, "w3T": w3T, "w2T": w2T,
        }
        opt = {"ln1_gb": [("ln1_g", ln1_g), ("ln1_b", ln1_b)],
               "qn_gb": [("qn_g", qn_g), ("qn_b", qn_b)],
               "kn_gb": [("kn_g", kn_g), ("kn_b", kn_b)],
               "ln2_gb": [("ln2_g", ln2_g), ("ln2_b", ln2_b)],
               "bqkv": [("b_qkv", b_qkv)], "bout": [("b_out", b_out)],
               "b1": [("b1", b1)], "b2": [("b2", b2)], "b3": [("b3", b3)]}
        for fl, items in opt.items():
            if fl in flags:
                for name, arr in items:
                    m[name] = np.ascontiguousarray(arr).astype(np.float32)
        in_maps.append(m)
    return in_maps, frozenset(flags), T, D


def kernel(**inputs):
    x = inputs["x"]
    B, S, D = x.shape
    in_maps, flags, T, _ = _prep_inputs(**inputs)

    key = (T, D, flags)
    if key not in _CACHE:
        _CACHE[key] = build_nc(T=T, D=D, flags=flags)
    nc = _CACHE[key]

    res = run_bass_kernel_spmd(nc, in_maps, core_ids=list(range(N_CORES)))
    out = np.empty((B * S, D), np.float32)
    for c in range(N_CORES):
        out[c * T:(c + 1) * T] = res.results[c]["out"]
    return out.reshape(B, S, D)
